# revision 15
# baseline (speedup 1.0000x reference)
"""Fused multi-head causal attention (RoPE) for Trainium2, 8-core SPMD.

Sharding: data-parallel over batch (B=2) x tensor-parallel over heads
(16 heads -> 4 per core, Megatron-style column/row split of the qkv/out
projections). Each core computes a partial (T, C) output; the host sums
the 4 partials per batch element.

v2 schedule (vs v1): attention for qb=1 (queries 512:1023, keys 0:1023
-- fully computable from the first token half) is interleaved into the
QKV projection of both halves so the Scalar engine's exp work starts
~40us earlier; remaining query blocks run in order 3,2,0 so the short
qb=0 block forms the tail.  Causal-mask adds stream a single [128,128]
tril tile (128 cols/diag-block instead of 512-dl).  Softmax
normalization is fused per head-pair: one reciprocal over [1,1024], one
K=1 broadcast matmul (fp32->f32r bitcast, no copy), and the normalize
multiply writes straight into the aou tiles (no intermediate DMA).
Output is stored bf16 (host accumulates in fp32).
"""

import sys
import numpy as np

if '/opt/trn_rl_repo' not in sys.path:
    sys.path.insert(0, '/opt/trn_rl_repo')

import ml_dtypes

B, T, C, H, D = 2, 2048, 1024, 16, 64
HPC = 4            # heads per core
NCORES = 8
NEG = -1.0e9
BF = ml_dtypes.bfloat16

_cache = {}


def _build(debug=False):
    import concourse.mybir as mybir
    from concourse import bacc
    import concourse.tile as tile

    F32 = mybir.dt.float32
    FR = mybir.dt.float32r
    B16 = mybir.dt.bfloat16
    AF = mybir.ActivationFunctionType

    nc = bacc.Bacc("TRN2", debug=False, enable_asserts=True)
    xT = nc.dram_tensor("xT", [C, T], B16, kind="ExternalInput").ap()
    wqk = nc.dram_tensor("wqk", [C, 512], B16, kind="ExternalInput").ap()
    wv = nc.dram_tensor("wv", [C, 256], B16, kind="ExternalInput").ap()
    wo = nc.dram_tensor("wo", [256, 1024], B16, kind="ExternalInput").ap()
    cosR = nc.dram_tensor("cosR", [128, T], B16, kind="ExternalInput").ap()
    sinS = nc.dram_tensor("sinS", [128, T], B16, kind="ExternalInput").ap()
    maskT = nc.dram_tensor("maskT", [128, 128], B16, kind="ExternalInput").ap()
    ident = nc.dram_tensor("ident", [128, 128], B16, kind="ExternalInput").ap()
    onesI = nc.dram_tensor("onesI", [65, 64], FR, kind="ExternalInput").ap()
    out = nc.dram_tensor("out", [T, C], B16, kind="ExternalOutput").ap()
    if debug:
        d_rot = [nc.dram_tensor(f"d_rot{i}", [128, T], B16, kind="ExternalOutput").ap()
                 for i in range(4)]
        d_v = nc.dram_tensor("d_v", [128, 16 * HPC * 65], B16, kind="ExternalOutput").ap()
        d_aou = [nc.dram_tensor(f"d_aou{i}", [128, T], B16, kind="ExternalOutput").ap()
                 for i in range(2)]
        d_stage = nc.dram_tensor("d_stage", [65, 1024], F32, kind="ExternalOutput").ap()
        d_rr = nc.dram_tensor("d_rr", [1, 1024], F32, kind="ExternalOutput").ap()
        d_rrr = nc.dram_tensor("d_rrr", [1, 1024], F32, kind="ExternalOutput").ap()

    NQ = 2            # token halves for streaming x
    QT = T // NQ      # 1024 tokens per half

    with tile.TileContext(nc) as tc:
        with tc.tile_pool(name="persist", bufs=1) as pp, \
             tc.tile_pool(name="rawp", bufs=3) as rawp, \
             tc.tile_pool(name="swpp", bufs=4) as swpp, \
             tc.tile_pool(name="ptp", bufs=3) as ptp, \
             tc.tile_pool(name="stagep", bufs=4) as stagep, \
             tc.tile_pool(name="rrp", bufs=4) as rrp, \
             tc.tile_pool(name="ysbp", bufs=2) as ysbp, \
             tc.tile_pool(name="xqp", bufs=2) as xqp, \
             tc.tile_pool(name="psA", bufs=3, space="PSUM") as psA, \
             tc.tile_pool(name="psO", bufs=2, space="PSUM") as psO:

            # ---- persistent SBUF tensors ----
            wqk_sb = pp.tile([128, 8 * 512], B16, tag="wqk")
            wv_sb = pp.tile([128, 8 * 256], B16, tag="wv")
            wo_sb = pp.tile([128, 2 * 1024], B16, tag="wo")
            cos_sb = pp.tile([128, T], B16, tag="cos")
            sin_sb = pp.tile([128, T], B16, tag="sin")
            maskT_sb = pp.tile([128, 128], B16, tag="maskT")
            id_sb = pp.tile([128, 128], B16, tag="id")
            ones_sb = pp.tile([65, 64], FR, tag="ones")
            v_sb = pp.tile([128, 16 * HPC * 65], B16, tag="v")
            qk_rot = [pp.tile([128, T], B16, tag=f"rot{i}", name=f"rot{i}") for i in range(4)]
            aou = [pp.tile([128, T], B16, tag=f"aou{i}", name=f"aou{i}") for i in range(2)]

            # initial loads: interleave per k-chunk so accumulation chains
            # can begin as chunks arrive
            xq_tiles = {}
            xq_tiles[0] = xqp.tile([128, 8 * QT], B16, tag="xq", name="xq0")
            for k in range(8):
                nc.sync.dma_start(wqk_sb[:, k * 512:(k + 1) * 512], wqk[k * 128:(k + 1) * 128, :])
                nc.sync.dma_start(wv_sb[:, k * 256:(k + 1) * 256], wv[k * 128:(k + 1) * 128, :])
                nc.sync.dma_start(xq_tiles[0][:, k * QT:(k + 1) * QT],
                                  xT[k * 128:(k + 1) * 128, 0:QT])
            nc.sync.dma_start(cos_sb[:], cosR[:])
            nc.sync.dma_start(sin_sb[:], sinS[:])
            nc.sync.dma_start(ones_sb[:], onesI[:])
            nc.sync.dma_start(maskT_sb[:], maskT[:])
            nc.sync.dma_start(id_sb[:], ident[:])
            # ones column at col 64 of every v slot
            vcols = v_sb.rearrange("p (s d) -> p s d", d=65)
            nc.vector.memset(vcols[:, :, 64:65], 1.0)

            # ---------- QKV projection + RoPE ----------
            def qk_chain(q, m):
                t0 = q * QT
                xq_sb = xq_tiles[q]
                ps = psA.tile([128, 1024], F32, tag="grp", name=f"qk{q}_{m}")
                for n in range(2):
                    for k in range(8):
                        nc.tensor.matmul(ps[:, n * 512:(n + 1) * 512],
                                         wqk_sb[:, k * 512 + m * 128: k * 512 + (m + 1) * 128],
                                         xq_sb[:, k * QT + n * 512: k * QT + (n + 1) * 512],
                                         start=(k == 0), stop=(k == 7))
                raw = rawp.tile([128, QT], B16, tag="raw")
                nc.scalar.copy(raw[:], ps[:])
                swp = swpp.tile([128, QT], B16, tag="swp")
                nc.gpsimd.dma_start(swp[0:32, :], raw[32:64, :])
                nc.gpsimd.dma_start(swp[32:64, :], raw[0:32, :])
                nc.gpsimd.dma_start(swp[64:96, :], raw[96:128, :])
                nc.gpsimd.dma_start(swp[96:128, :], raw[64:96, :])
                rot = qk_rot[m]
                cs = cos_sb[:, t0:t0 + QT]
                sn = sin_sb[:, t0:t0 + QT]
                tmp = swpp.tile([128, QT], B16, tag="tmp")
                nc.vector.tensor_mul(tmp[:], swp[:], sn)
                nc.vector.tensor_mul(rot[:, t0:t0 + QT], raw[:], cs)
                nc.vector.tensor_add(rot[:, t0:t0 + QT], rot[:, t0:t0 + QT], tmp[:])

            def v_pair(q, j):
                # v for kb = q*8 + 2j, q*8 + 2j + 1
                xq_sb = xq_tiles[q]
                ps = psA.tile([128, 512], F32, tag="grp", name=f"v{q}_{j}")
                for i, mt in enumerate((2 * j, 2 * j + 1)):
                    for k in range(8):
                        nc.tensor.matmul(ps[:, i * 256:(i + 1) * 256],
                                         xq_sb[:, k * QT + mt * 128: k * QT + (mt + 1) * 128],
                                         wv_sb[:, k * 256:(k + 1) * 256],
                                         start=(k == 0), stop=(k == 7))
                vsrc = ps.rearrange("p (s h d) -> p s h d", s=2, h=4)
                base = (q * 8 + 2 * j) * 4 * 65
                vdst = v_sb[:, base:base + 2 * 4 * 65] \
                    .rearrange("p (s h e) -> p s h e", s=2, e=65)[:, :, :, 0:64]
                nc.scalar.copy(vdst, vsrc)

            # ---------- attention ----------
            def st_group(qb, h, g, st):
                h2 = h % 2
                Qt = qk_rot[0] if h < 2 else qk_rot[1]
                Kt = qk_rot[2] if h < 2 else qk_rot[3]
                r0 = 64 * h2
                for i in range(2):
                    kb = 2 * g + i
                    kt = Kt[r0:r0 + 64, kb * 128:(kb + 1) * 128]
                    if kb < 4 * qb:
                        nc.tensor.matmul(
                            st[:, i * 512:(i + 1) * 512], kt,
                            Qt[r0:r0 + 64, qb * 512:(qb + 1) * 512],
                            start=True, stop=True)
                    else:
                        dl = (kb - 4 * qb) * 128
                        if dl < 384:
                            nc.tensor.matmul(
                                st[:, i * 512 + dl + 128:(i + 1) * 512], kt,
                                Qt[r0:r0 + 64, qb * 512 + dl + 128:(qb + 1) * 512],
                                start=True, stop=True)
                        nc.tensor.matmul(
                            st[:, i * 512 + dl: i * 512 + dl + 128], kt,
                            Qt[r0:r0 + 64, qb * 512 + dl: qb * 512 + dl + 128],
                            start=True, stop=False)
                for i in range(2):
                    kb = 2 * g + i
                    if kb >= 4 * qb:
                        dl = (kb - 4 * qb) * 128
                        nc.tensor.matmul(
                            st[:, i * 512 + dl: i * 512 + dl + 128],
                            id_sb[:], maskT_sb[:],
                            start=False, stop=True)

            def pv_group(qb, h, g, pt, out_ps, live):
                for i in range(2):
                    kb = 2 * g + i
                    diag = kb >= 4 * qb
                    dl = (kb - 4 * qb) * 128 if diag else 0
                    nc.tensor.matmul(
                        out_ps[:, dl:512],
                        v_sb[:, (kb * 4 + h) * 65:(kb * 4 + h) * 65 + 65],
                        pt[:, i * 512 + dl:(i + 1) * 512],
                        start=(kb == 0), stop=(kb == live - 1))

            norm_state = {}

            def att_block(qb, hp, hpi):
                """Generator: one yield per g-step; ends after norm1."""
                live = 4 * (qb + 1)
                ng = live // 2
                out_ps = {}
                pts = {h: {} for h in hp}
                for g in range(ng + 1):
                    for h in hp:
                        if g < ng:
                            st = psA.tile([128, 1024], F32, tag="grp",
                                          name=f"st{qb}_{h}_{g}")
                            st_group(qb, h, g, st)
                            if g < 2 * qb:
                                ranges = ((0, 1024),)
                            elif g == 2 * qb:
                                ranges = ((0, 512), (640, 1024))
                            else:
                                ranges = ((256, 512), (896, 1024))
                            pt = ptp.tile([128, 1024], B16, tag="pt",
                                          name=f"pt{qb}_{h}_{g}")
                            for lo, hi in ranges:
                                nc.scalar.activation(pt[:, lo:hi], st[:, lo:hi],
                                                     AF.Exp, scale=0.125)
                            pts[h][g] = pt
                    for h in hp:
                        if g >= 1:
                            if h not in out_ps:
                                out_ps[h] = psO.tile([65, 512], F32, tag="acc",
                                                     name=f"ops{qb}_{h}")
                            pv_group(qb, h, g - 1, pts[h].pop(g - 1),
                                     out_ps[h], live)
                    if g < ng:
                        yield
                # norm part 1: stage both heads side by side; one reciprocal
                stage = stagep.tile([65, 1024], F32, tag="stage",
                                    name=f"stage{qb}_{hpi}")
                nc.vector.tensor_copy(stage[:, 0:512], out_ps[hp[0]][:])
                nc.vector.tensor_copy(stage[:, 512:1024], out_ps[hp[1]][:])
                den0 = rrp.tile([1, 1024], F32, tag="den", name=f"den{qb}_{hpi}")
                nc.gpsimd.dma_start(den0[:], stage[64:65, :])
                rr = rrp.tile([1, 1024], F32, tag="rr", name=f"rr{qb}_{hpi}")
                rrr = rrp.tile([1, 1024], FR, tag="rrr", name=f"rrr{qb}_{hpi}")
                with nc.allow_low_precision(reason="softmax denominators"):
                    nc.vector.reciprocal_approx_fast(rr[:], den0[:])
                    nc.gpsimd.tensor_copy(rrr[:], rr[:])
                if debug and qb == 0 and hpi == 0:
                    nc.sync.dma_start(d_stage[:], stage[:])
                    nc.sync.dma_start(d_rr[:], rr[:])
                    nc.sync.dma_start(d_rrr[:], rrr[:].bitcast(F32))
                norm_state[(qb, hpi)] = (stage, rrr)
                yield

            def norm2(qb, hpi):
                stage, rrr = norm_state.pop((qb, hpi))
                bc = psA.tile([64, 1024], F32, tag="grp", name=f"bc{qb}_{hpi}")
                for i in range(2):
                    nc.tensor.matmul(bc[:, i * 512:(i + 1) * 512],
                                     ones_sb[0:1, 0:64],
                                     rrr[0:1, i * 512:(i + 1) * 512],
                                     start=True, stop=True)
                AO = aou[hpi]
                nc.vector.tensor_mul(AO[0:64, qb * 512:(qb + 1) * 512],
                                     stage[0:64, 0:512], bc[:, 0:512])
                nc.vector.tensor_mul(AO[64:128, qb * 512:(qb + 1) * 512],
                                     stage[0:64, 512:1024], bc[:, 512:1024])

            def outproj_qt(qt):
                yp = psA.tile([128, 1024], F32, tag="grp", name=f"yp{qt}")
                for nh in range(2):
                    nc.tensor.matmul(yp[:, nh * 512:(nh + 1) * 512],
                                     aou[0][:, qt * 128:(qt + 1) * 128],
                                     wo_sb[:, nh * 512:(nh + 1) * 512],
                                     start=True, stop=False)
                    nc.tensor.matmul(yp[:, nh * 512:(nh + 1) * 512],
                                     aou[1][:, qt * 128:(qt + 1) * 128],
                                     wo_sb[:, 1024 + nh * 512:1024 + (nh + 1) * 512],
                                     start=False, stop=True)
                ysb = ysbp.tile([128, 1024], B16, tag="y", name=f"ysb{qt}")
                nc.vector.tensor_copy(ysb[:], yp[:])
                nc.sync.dma_start(out[qt * 128:(qt + 1) * 128, :], ysb[:])

            # ================= schedule =================
            # --- phase 1: QKV half 0, attention qb=1 hp0 interleaved
            gen0 = att_block(1, (0, 1), 0)
            qk_chain(0, 0)
            qk_chain(0, 2)
            for j in range(4):
                v_pair(0, j)
                next(gen0, None)
            qk_chain(0, 1)
            # prefetch x half 1 + deferred weights
            xq_tiles[1] = xqp.tile([128, 8 * QT], B16, tag="xq", name="xq1")
            for k in range(8):
                nc.sync.dma_start(xq_tiles[1][:, k * QT:(k + 1) * QT],
                                  xT[k * 128:(k + 1) * 128, QT:T])
            for k in range(2):
                nc.sync.dma_start(wo_sb[:, k * 1024:(k + 1) * 1024], wo[k * 128:(k + 1) * 128, :])
            for _ in gen0:
                pass
            qk_chain(0, 3)

            # --- phase 2: QKV half 1, attention qb=1 hp1 interleaved
            gen1 = att_block(1, (2, 3), 1)
            qk_chain(1, 0)
            next(gen1, None)
            qk_chain(1, 2)
            next(gen1, None)
            for j in range(4):
                v_pair(1, j)
                next(gen1, None)
            qk_chain(1, 1)
            for _ in gen1:
                pass
            qk_chain(1, 3)

            # --- remaining query blocks: 3, 2, 0; outproj/norm2 one qb late
            def run_block(qb, fillers):
                fillers = list(fillers)
                for hpi, hp in enumerate(((0, 1), (2, 3))):
                    for _ in att_block(qb, hp, hpi):
                        if fillers:
                            fillers.pop(0)()
                for f in fillers:
                    f()

            def mk_fillers(pqb):
                fs = [lambda: norm2(pqb, 0), lambda: norm2(pqb, 1)]
                fs += [(lambda qt=qt: outproj_qt(qt)) for qt in range(4 * pqb, 4 * pqb + 4)]
                return fs

            run_block(3, mk_fillers(1))
            run_block(2, mk_fillers(3))
            run_block(0, mk_fillers(2))
            norm2(0, 0)
            norm2(0, 1)
            for qt in range(0, 4):
                outproj_qt(qt)
            if debug:
                for i in range(4):
                    nc.sync.dma_start(d_rot[i][:], qk_rot[i][:])
                nc.sync.dma_start(d_v[:], v_sb[:])
                for i in range(2):
                    nc.sync.dma_start(d_aou[i][:], aou[i][:])

    nc.compile()
    return nc


def _core_inputs(x, cos, sin, W_qkv, W_out, core):
    b = core // 4
    hg = core % 4
    heads = list(range(4 * hg, 4 * hg + 4))

    xT = np.ascontiguousarray(x[b].T).astype(BF)
    qrows = np.concatenate([W_qkv[h * 64:(h + 1) * 64] for h in heads], 0)
    krows = np.concatenate([W_qkv[C + h * 64: C + (h + 1) * 64] for h in heads], 0)
    wqk = np.ascontiguousarray(np.concatenate([qrows, krows], 0).T).astype(BF)
    vrows = np.concatenate([W_qkv[2 * C + h * 64: 2 * C + (h + 1) * 64] for h in heads], 0)
    wv = np.ascontiguousarray(vrows.T).astype(BF)
    cols = np.concatenate([np.arange(h * 64, (h + 1) * 64) for h in heads])
    wo = np.ascontiguousarray(W_out[:, cols].T).astype(BF)

    cT = np.ascontiguousarray(cos.T)      # (32, T)
    sT = np.ascontiguousarray(sin.T)
    cosR = np.tile(cT, (4, 1)).astype(BF)
    sinS = np.concatenate([-sT, sT, -sT, sT], 0).astype(BF)

    p = np.arange(128)[:, None]
    j = np.arange(128)[None, :]
    maskT = np.where(p <= j, 0.0, NEG).astype(BF)

    return {
        "xT": xT, "wqk": wqk, "wv": wv, "wo": wo,
        "cosR": cosR, "sinS": sinS,
        "maskT": np.ascontiguousarray(maskT),
        "ident": np.eye(128).astype(BF),
        "onesI": np.ones((65, 64), dtype=np.float32),
    }


def kernel(x, cos, sin, mask, W_qkv, W_out):
    from concourse import bass_utils

    x = np.asarray(x, dtype=np.float32)
    cos = np.asarray(cos, dtype=np.float32)
    sin = np.asarray(sin, dtype=np.float32)
    W_qkv = np.asarray(W_qkv, dtype=np.float32)
    W_out = np.asarray(W_out, dtype=np.float32)

    if "nc" not in _cache:
        _cache["nc"] = _build()
    nc = _cache["nc"]

    in_maps = [_core_inputs(x, cos, sin, W_qkv, W_out, c) for c in range(NCORES)]
    res = bass_utils.run_bass_kernel_spmd(nc, in_maps, core_ids=list(range(NCORES)))

    y = np.zeros((B, T, C), dtype=np.float32)
    for c in range(NCORES):
        y[c // 4] += res.results[c]["out"].astype(np.float32)
    return y


# revision 20
# speedup vs baseline: 1.1159x; 1.1159x over previous
"""Fused multi-head causal attention (RoPE) for Trainium2, 8-core SPMD.

Sharding: data-parallel over batch (B=2) x tensor-parallel over heads
(16 heads -> 4 per core, Megatron-style column/row split of the qkv/out
projections). Each core computes a partial (T, C) output; the host sums
the 4 partials per batch element.

v2 schedule (vs v1): attention for qb=1 (queries 512:1023, keys 0:1023
-- fully computable from the first token half) is interleaved into the
QKV projection of both halves so the Scalar engine's exp work starts
~40us earlier; remaining query blocks run in order 3,2,0 so the short
qb=0 block forms the tail.  Causal-mask adds stream a single [128,128]
tril tile (128 cols/diag-block instead of 512-dl).  Softmax
normalization is fused per head-pair: one reciprocal over [1,1024], one
K=1 broadcast matmul (fp32->f32r bitcast, no copy), and the normalize
multiply writes straight into the aou tiles (no intermediate DMA).
Output is stored bf16 (host accumulates in fp32).
"""

import sys
import numpy as np

if '/opt/trn_rl_repo' not in sys.path:
    sys.path.insert(0, '/opt/trn_rl_repo')

import ml_dtypes

B, T, C, H, D = 2, 2048, 1024, 16, 64
HPC = 4            # heads per core
NCORES = 8
NEG = -1.0e9
BF = ml_dtypes.bfloat16

_cache = {}


def _build(debug=False):
    import concourse.mybir as mybir
    from concourse import bacc
    import concourse.tile as tile

    F32 = mybir.dt.float32
    FR = mybir.dt.float32r
    B16 = mybir.dt.bfloat16
    AF = mybir.ActivationFunctionType

    nc = bacc.Bacc("TRN2", debug=False, enable_asserts=True)
    xT = nc.dram_tensor("xT", [C, T], B16, kind="ExternalInput").ap()
    wqk = nc.dram_tensor("wqk", [C, 512], B16, kind="ExternalInput").ap()
    wv = nc.dram_tensor("wv", [C, 256], B16, kind="ExternalInput").ap()
    wo = nc.dram_tensor("wo", [256, 1024], B16, kind="ExternalInput").ap()
    cosR = nc.dram_tensor("cosR", [128, T], B16, kind="ExternalInput").ap()
    sinS = nc.dram_tensor("sinS", [128, T], B16, kind="ExternalInput").ap()
    maskT = nc.dram_tensor("maskT", [128, 128], B16, kind="ExternalInput").ap()
    ident = nc.dram_tensor("ident", [128, 128], B16, kind="ExternalInput").ap()
    onesI = nc.dram_tensor("onesI", [65, 64], FR, kind="ExternalInput").ap()
    out = nc.dram_tensor("out", [T, C], B16, kind="ExternalOutput").ap()
    if debug:
        d_rot = [nc.dram_tensor(f"d_rot{i}", [128, T], B16, kind="ExternalOutput").ap()
                 for i in range(4)]
        d_v = nc.dram_tensor("d_v", [128, 16 * HPC * 65], B16, kind="ExternalOutput").ap()
        d_aou = [nc.dram_tensor(f"d_aou{i}", [128, T], B16, kind="ExternalOutput").ap()
                 for i in range(2)]
        d_stage = nc.dram_tensor("d_stage", [65, 1024], F32, kind="ExternalOutput").ap()
        d_rr = nc.dram_tensor("d_rr", [1, 1024], F32, kind="ExternalOutput").ap()
        d_rrr = nc.dram_tensor("d_rrr", [1, 1024], F32, kind="ExternalOutput").ap()

    NQ = 2            # token halves for streaming x
    QT = T // NQ      # 1024 tokens per half

    with tile.TileContext(nc) as tc:
        with tc.tile_pool(name="persist", bufs=1) as pp, \
             tc.tile_pool(name="rawp", bufs=3) as rawp, \
             tc.tile_pool(name="swpp", bufs=4) as swpp, \
             tc.tile_pool(name="ptp", bufs=3) as ptp, \
             tc.tile_pool(name="stagep", bufs=4) as stagep, \
             tc.tile_pool(name="rrp", bufs=4) as rrp, \
             tc.tile_pool(name="ysbp", bufs=2) as ysbp, \
             tc.tile_pool(name="xqp", bufs=2) as xqp, \
             tc.tile_pool(name="psA", bufs=3, space="PSUM") as psA, \
             tc.tile_pool(name="psO", bufs=2, space="PSUM") as psO:

            # ---- persistent SBUF tensors ----
            wqk_sb = pp.tile([128, 8 * 512], B16, tag="wqk")
            wv_sb = pp.tile([128, 8 * 256], B16, tag="wv")
            wo_sb = pp.tile([128, 2 * 1024], B16, tag="wo")
            cos_sb = pp.tile([128, T], B16, tag="cos")
            sin_sb = pp.tile([128, T], B16, tag="sin")
            maskT_sb = pp.tile([128, 128], B16, tag="maskT")
            id_sb = pp.tile([128, 128], B16, tag="id")
            ones_sb = pp.tile([65, 64], FR, tag="ones")
            v_sb = pp.tile([128, 16 * HPC * 65], B16, tag="v")
            qk_rot = [pp.tile([128, T], B16, tag=f"rot{i}", name=f"rot{i}") for i in range(4)]
            aou = [pp.tile([128, T], B16, tag=f"aou{i}", name=f"aou{i}") for i in range(2)]

            # initial loads: interleave per k-chunk so accumulation chains
            # can begin as chunks arrive
            # batched loads: ONE DMA per tensor (sync-queue issue rate is
            # ~0.65us/instruction, so chunked loads serialize on issue)
            xq_tiles = {}
            xq_tiles[0] = xqp.tile([128, 8 * QT], B16, tag="xq", name="xq0")
            nc.sync.dma_start(wqk_sb.rearrange("p (k c) -> p k c", k=8),
                              wqk.rearrange("(k p) c -> p k c", p=128))
            nc.sync.dma_start(wv_sb.rearrange("p (k c) -> p k c", k=8),
                              wv.rearrange("(k p) c -> p k c", p=128))
            nc.sync.dma_start(xq_tiles[0].rearrange("p (k c) -> p k c", k=8),
                              xT[:, 0:QT].rearrange("(k p) c -> p k c", p=128))
            nc.sync.dma_start(cos_sb[:], cosR[:])
            nc.sync.dma_start(sin_sb[:], sinS[:])
            nc.sync.dma_start(ones_sb[:], onesI[:])
            nc.sync.dma_start(maskT_sb[:], maskT[:])
            nc.sync.dma_start(id_sb[:], ident[:])
            # ones column at col 64 of every v slot
            vcols = v_sb.rearrange("p (s d) -> p s d", d=65)
            nc.vector.memset(vcols[:, :, 64:65], 1.0)

            # ---------- QKV projection + RoPE ----------
            def qk_chain(q, m):
                t0 = q * QT
                xq_sb = xq_tiles[q]
                ps = psA.tile([128, 1024], F32, tag="grp", name=f"qk{q}_{m}")
                for n in range(2):
                    for k in range(8):
                        nc.tensor.matmul(ps[:, n * 512:(n + 1) * 512],
                                         wqk_sb[:, k * 512 + m * 128: k * 512 + (m + 1) * 128],
                                         xq_sb[:, k * QT + n * 512: k * QT + (n + 1) * 512],
                                         start=(k == 0), stop=(k == 7))
                raw = rawp.tile([128, QT], B16, tag="raw")
                nc.scalar.copy(raw[:], ps[:])
                swp = swpp.tile([128, QT], B16, tag="swp")
                nc.sync.dma_start(swp[0:32, :], raw[32:64, :])
                nc.sync.dma_start(swp[32:64, :], raw[0:32, :])
                nc.sync.dma_start(swp[64:96, :], raw[96:128, :])
                nc.sync.dma_start(swp[96:128, :], raw[64:96, :])
                rot = qk_rot[m]
                cs = cos_sb[:, t0:t0 + QT]
                sn = sin_sb[:, t0:t0 + QT]
                tmp = swpp.tile([128, QT], B16, tag="tmp")
                nc.vector.tensor_mul(tmp[:], swp[:], sn)
                nc.vector.tensor_mul(rot[:, t0:t0 + QT], raw[:], cs)
                nc.vector.tensor_add(rot[:, t0:t0 + QT], rot[:, t0:t0 + QT], tmp[:])

            def v_pair(q, j):
                # v for kb = q*8 + 2j, q*8 + 2j + 1
                xq_sb = xq_tiles[q]
                ps = psA.tile([128, 512], F32, tag="grp", name=f"v{q}_{j}")
                for i, mt in enumerate((2 * j, 2 * j + 1)):
                    for k in range(8):
                        nc.tensor.matmul(ps[:, i * 256:(i + 1) * 256],
                                         xq_sb[:, k * QT + mt * 128: k * QT + (mt + 1) * 128],
                                         wv_sb[:, k * 256:(k + 1) * 256],
                                         start=(k == 0), stop=(k == 7))
                vsrc = ps.rearrange("p (s h d) -> p s h d", s=2, h=4)
                base = (q * 8 + 2 * j) * 4 * 65
                vdst = v_sb[:, base:base + 2 * 4 * 65] \
                    .rearrange("p (s h e) -> p s h e", s=2, e=65)[:, :, :, 0:64]
                nc.scalar.copy(vdst, vsrc)

            # ---------- attention ----------
            def st_group(qb, h, g, st):
                h2 = h % 2
                Qt = qk_rot[0] if h < 2 else qk_rot[1]
                Kt = qk_rot[2] if h < 2 else qk_rot[3]
                r0 = 64 * h2
                for i in range(2):
                    kb = 2 * g + i
                    kt = Kt[r0:r0 + 64, kb * 128:(kb + 1) * 128]
                    if kb < 4 * qb:
                        nc.tensor.matmul(
                            st[:, i * 512:(i + 1) * 512], kt,
                            Qt[r0:r0 + 64, qb * 512:(qb + 1) * 512],
                            start=True, stop=True)
                    else:
                        dl = (kb - 4 * qb) * 128
                        if dl < 384:
                            nc.tensor.matmul(
                                st[:, i * 512 + dl + 128:(i + 1) * 512], kt,
                                Qt[r0:r0 + 64, qb * 512 + dl + 128:(qb + 1) * 512],
                                start=True, stop=True)
                        nc.tensor.matmul(
                            st[:, i * 512 + dl: i * 512 + dl + 128], kt,
                            Qt[r0:r0 + 64, qb * 512 + dl: qb * 512 + dl + 128],
                            start=True, stop=False)
                for i in range(2):
                    kb = 2 * g + i
                    if kb >= 4 * qb:
                        dl = (kb - 4 * qb) * 128
                        nc.tensor.matmul(
                            st[:, i * 512 + dl: i * 512 + dl + 128],
                            id_sb[:], maskT_sb[:],
                            start=False, stop=True)

            def pv_group(qb, h, g, pt, out_ps, live):
                for i in range(2):
                    kb = 2 * g + i
                    diag = kb >= 4 * qb
                    dl = (kb - 4 * qb) * 128 if diag else 0
                    nc.tensor.matmul(
                        out_ps[:, dl:512],
                        v_sb[:, (kb * 4 + h) * 65:(kb * 4 + h) * 65 + 65],
                        pt[:, i * 512 + dl:(i + 1) * 512],
                        start=(kb == 0), stop=(kb == live - 1))

            norm_state = {}

            def att_block(qb, hp, hpi):
                """Generator: one yield per g-step; ends after norm1."""
                live = 4 * (qb + 1)
                ng = live // 2
                out_ps = {}
                pts = {h: {} for h in hp}
                for g in range(ng + 1):
                    for h in hp:
                        if g < ng:
                            st = psA.tile([128, 1024], F32, tag="grp",
                                          name=f"st{qb}_{h}_{g}")
                            st_group(qb, h, g, st)
                            if g < 2 * qb:
                                ranges = ((0, 1024),)
                            elif g == 2 * qb:
                                ranges = ((0, 512), (640, 1024))
                            else:
                                ranges = ((256, 512), (896, 1024))
                            pt = ptp.tile([128, 1024], B16, tag="pt",
                                          name=f"pt{qb}_{h}_{g}")
                            for lo, hi in ranges:
                                nc.scalar.activation(pt[:, lo:hi], st[:, lo:hi],
                                                     AF.Exp, scale=0.125)
                            pts[h][g] = pt
                    for h in hp:
                        if g >= 1:
                            if h not in out_ps:
                                out_ps[h] = psO.tile([65, 512], F32, tag="acc",
                                                     name=f"ops{qb}_{h}")
                            pv_group(qb, h, g - 1, pts[h].pop(g - 1),
                                     out_ps[h], live)
                    if g < ng:
                        yield
                # norm part 1: stage both heads side by side; one reciprocal
                stage = stagep.tile([65, 1024], F32, tag="stage",
                                    name=f"stage{qb}_{hpi}")
                nc.vector.tensor_copy(stage[:, 0:512], out_ps[hp[0]][:])
                nc.vector.tensor_copy(stage[:, 512:1024], out_ps[hp[1]][:])
                den0 = rrp.tile([1, 1024], F32, tag="den", name=f"den{qb}_{hpi}")
                nc.sync.dma_start(den0[:], stage[64:65, :])
                rr = rrp.tile([1, 1024], F32, tag="rr", name=f"rr{qb}_{hpi}")
                rrr = rrp.tile([1, 1024], FR, tag="rrr", name=f"rrr{qb}_{hpi}")
                with nc.allow_low_precision(reason="softmax denominators"):
                    nc.vector.reciprocal_approx_fast(rr[:], den0[:])
                    nc.vector.tensor_copy(rrr[:], rr[:])
                if debug and qb == 0 and hpi == 0:
                    nc.sync.dma_start(d_stage[:], stage[:])
                    nc.sync.dma_start(d_rr[:], rr[:])
                    nc.sync.dma_start(d_rrr[:], rrr[:].bitcast(F32))
                norm_state[(qb, hpi)] = (stage, rrr)
                yield

            def norm2(qb, hpi):
                stage, rrr = norm_state.pop((qb, hpi))
                bc = psA.tile([64, 1024], F32, tag="grp", name=f"bc{qb}_{hpi}")
                for i in range(2):
                    nc.tensor.matmul(bc[:, i * 512:(i + 1) * 512],
                                     ones_sb[0:1, 0:64],
                                     rrr[0:1, i * 512:(i + 1) * 512],
                                     start=True, stop=True)
                AO = aou[hpi]
                nc.vector.tensor_mul(AO[0:64, qb * 512:(qb + 1) * 512],
                                     stage[0:64, 0:512], bc[:, 0:512])
                nc.vector.tensor_mul(AO[64:128, qb * 512:(qb + 1) * 512],
                                     stage[0:64, 512:1024], bc[:, 512:1024])

            def outproj_qt(qt):
                yp = psA.tile([128, 1024], F32, tag="grp", name=f"yp{qt}")
                for nh in range(2):
                    nc.tensor.matmul(yp[:, nh * 512:(nh + 1) * 512],
                                     aou[0][:, qt * 128:(qt + 1) * 128],
                                     wo_sb[:, nh * 512:(nh + 1) * 512],
                                     start=True, stop=False)
                    nc.tensor.matmul(yp[:, nh * 512:(nh + 1) * 512],
                                     aou[1][:, qt * 128:(qt + 1) * 128],
                                     wo_sb[:, 1024 + nh * 512:1024 + (nh + 1) * 512],
                                     start=False, stop=True)
                ysb = ysbp.tile([128, 1024], B16, tag="y", name=f"ysb{qt}")
                nc.vector.tensor_copy(ysb[:], yp[:])
                nc.sync.dma_start(out[qt * 128:(qt + 1) * 128, :], ysb[:])

            # ================= schedule =================
            # --- phase 1: QKV half 0, attention qb=1 hp0 interleaved
            gen0 = att_block(1, (0, 1), 0)
            qk_chain(0, 0)
            qk_chain(0, 2)
            for j in range(4):
                v_pair(0, j)
                next(gen0, None)
            qk_chain(0, 1)
            # prefetch x half 1 + deferred weights
            xq_tiles[1] = xqp.tile([128, 8 * QT], B16, tag="xq", name="xq1")
            for k in range(8):
                nc.sync.dma_start(xq_tiles[1][:, k * QT:(k + 1) * QT],
                                  xT[k * 128:(k + 1) * 128, QT:T])
            for k in range(2):
                nc.sync.dma_start(wo_sb[:, k * 1024:(k + 1) * 1024], wo[k * 128:(k + 1) * 128, :])
            for _ in gen0:
                pass
            qk_chain(0, 3)

            # --- phase 2: QKV half 1, attention qb=1 hp1 interleaved
            gen1 = att_block(1, (2, 3), 1)
            qk_chain(1, 0)
            next(gen1, None)
            qk_chain(1, 2)
            next(gen1, None)
            for j in range(4):
                v_pair(1, j)
                next(gen1, None)
            qk_chain(1, 1)
            for _ in gen1:
                pass
            qk_chain(1, 3)

            # --- remaining query blocks: 3, 2, 0; outproj/norm2 one qb late
            def run_block(qb, fillers):
                # delay fillers a couple of g-steps so the bc matmul (which
                # waits on the previous block's reciprocal chain) never heads
                # the PE queue while early st-groups could run
                fillers = list(fillers)
                total = 2 * (2 * (qb + 1) + 1)
                skip = max(0, min(2, total - len(fillers)))
                step = 0
                for hpi, hp in enumerate(((0, 1), (2, 3))):
                    for _ in att_block(qb, hp, hpi):
                        if fillers and step >= skip:
                            fillers.pop(0)()
                        step += 1
                for f in fillers:
                    f()

            def mk_fillers(pqb):
                fs = [lambda: norm2(pqb, 0), lambda: norm2(pqb, 1)]
                fs += [(lambda qt=qt: outproj_qt(qt)) for qt in range(4 * pqb, 4 * pqb + 4)]
                return fs

            run_block(3, mk_fillers(1))
            run_block(2, mk_fillers(3))
            run_block(0, mk_fillers(2))
            norm2(0, 0)
            norm2(0, 1)
            for qt in range(0, 4):
                outproj_qt(qt)
            if debug:
                for i in range(4):
                    nc.sync.dma_start(d_rot[i][:], qk_rot[i][:])
                nc.sync.dma_start(d_v[:], v_sb[:])
                for i in range(2):
                    nc.sync.dma_start(d_aou[i][:], aou[i][:])

    nc.compile()
    return nc


def _core_inputs(x, cos, sin, W_qkv, W_out, core):
    b = core // 4
    hg = core % 4
    heads = list(range(4 * hg, 4 * hg + 4))

    xT = np.ascontiguousarray(x[b].T).astype(BF)
    qrows = np.concatenate([W_qkv[h * 64:(h + 1) * 64] for h in heads], 0)
    krows = np.concatenate([W_qkv[C + h * 64: C + (h + 1) * 64] for h in heads], 0)
    wqk = np.ascontiguousarray(np.concatenate([qrows, krows], 0).T).astype(BF)
    vrows = np.concatenate([W_qkv[2 * C + h * 64: 2 * C + (h + 1) * 64] for h in heads], 0)
    wv = np.ascontiguousarray(vrows.T).astype(BF)
    cols = np.concatenate([np.arange(h * 64, (h + 1) * 64) for h in heads])
    wo = np.ascontiguousarray(W_out[:, cols].T).astype(BF)

    cT = np.ascontiguousarray(cos.T)      # (32, T)
    sT = np.ascontiguousarray(sin.T)
    cosR = np.tile(cT, (4, 1)).astype(BF)
    sinS = np.concatenate([-sT, sT, -sT, sT], 0).astype(BF)

    p = np.arange(128)[:, None]
    j = np.arange(128)[None, :]
    maskT = np.where(p <= j, 0.0, NEG).astype(BF)

    return {
        "xT": xT, "wqk": wqk, "wv": wv, "wo": wo,
        "cosR": cosR, "sinS": sinS,
        "maskT": np.ascontiguousarray(maskT),
        "ident": np.eye(128).astype(BF),
        "onesI": np.ones((65, 64), dtype=np.float32),
    }


def kernel(x, cos, sin, mask, W_qkv, W_out):
    from concourse import bass_utils

    x = np.asarray(x, dtype=np.float32)
    cos = np.asarray(cos, dtype=np.float32)
    sin = np.asarray(sin, dtype=np.float32)
    W_qkv = np.asarray(W_qkv, dtype=np.float32)
    W_out = np.asarray(W_out, dtype=np.float32)

    if "nc" not in _cache:
        _cache["nc"] = _build()
    nc = _cache["nc"]

    in_maps = [_core_inputs(x, cos, sin, W_qkv, W_out, c) for c in range(NCORES)]
    res = bass_utils.run_bass_kernel_spmd(nc, in_maps, core_ids=list(range(NCORES)))

    y = np.zeros((B, T, C), dtype=np.float32)
    for c in range(NCORES):
        y[c // 4] += res.results[c]["out"].astype(np.float32)
    return y


# revision 31
# speedup vs baseline: 1.1530x; 1.0332x over previous
"""Fused multi-head causal attention (RoPE) for Trainium2, 8-core SPMD.

Sharding: data-parallel over batch (B=2) x tensor-parallel over heads
(16 heads -> 4 per core, Megatron-style column/row split of the qkv/out
projections). Each core computes a partial (T, C) output; the host sums
the 4 partials per batch element.

v2 schedule (vs v1): attention for qb=1 (queries 512:1023, keys 0:1023
-- fully computable from the first token half) is interleaved into the
QKV projection of both halves so the Scalar engine's exp work starts
~40us earlier; remaining query blocks run in order 3,2,0 so the short
qb=0 block forms the tail.  Causal-mask adds stream a single [128,128]
tril tile (128 cols/diag-block instead of 512-dl).  Softmax
normalization is fused per head-pair: one reciprocal over [1,1024], one
K=1 broadcast matmul (fp32->f32r bitcast, no copy), and the normalize
multiply writes straight into the aou tiles (no intermediate DMA).
Output is stored bf16 (host accumulates in fp32).
"""

import sys
import numpy as np

if '/opt/trn_rl_repo' not in sys.path:
    sys.path.insert(0, '/opt/trn_rl_repo')

import ml_dtypes

B, T, C, H, D = 2, 2048, 1024, 16, 64
HPC = 4            # heads per core
NCORES = 8
NEG = -1.0e9
BF = ml_dtypes.bfloat16

_cache = {}


def _build(debug=False):
    import concourse.mybir as mybir
    from concourse import bacc
    import concourse.tile as tile

    F32 = mybir.dt.float32
    FR = mybir.dt.float32r
    B16 = mybir.dt.bfloat16
    AF = mybir.ActivationFunctionType

    nc = bacc.Bacc("TRN2", debug=False, enable_asserts=True)
    xT = nc.dram_tensor("xT", [C, T], B16, kind="ExternalInput").ap()
    wqk = nc.dram_tensor("wqk", [C, 512], B16, kind="ExternalInput").ap()
    wv = nc.dram_tensor("wv", [C, 256], B16, kind="ExternalInput").ap()
    wo = nc.dram_tensor("wo", [256, 1024], B16, kind="ExternalInput").ap()
    cosR = nc.dram_tensor("cosR", [128, T], B16, kind="ExternalInput").ap()
    sinS = nc.dram_tensor("sinS", [128, T], B16, kind="ExternalInput").ap()
    maskT = nc.dram_tensor("maskT", [128, 128], B16, kind="ExternalInput").ap()
    ident = nc.dram_tensor("ident", [128, 128], B16, kind="ExternalInput").ap()
    out = nc.dram_tensor("out", [T, C], B16, kind="ExternalOutput").ap()
    if debug:
        d_rot = [nc.dram_tensor(f"d_rot{i}", [128, T], B16, kind="ExternalOutput").ap()
                 for i in range(4)]
        d_v = nc.dram_tensor("d_v", [128, 16 * HPC * 65], B16, kind="ExternalOutput").ap()
        d_aou = [nc.dram_tensor(f"d_aou{i}", [128, T], B16, kind="ExternalOutput").ap()
                 for i in range(2)]
        d_stage = nc.dram_tensor("d_stage", [65, 1024], F32, kind="ExternalOutput").ap()
        d_rr = nc.dram_tensor("d_rr", [1, 1024], F32, kind="ExternalOutput").ap()
        d_rrr = nc.dram_tensor("d_rrr", [1, 1024], F32, kind="ExternalOutput").ap()

    NQ = 2            # token halves for streaming x
    QT = T // NQ      # 1024 tokens per half

    with tile.TileContext(nc) as tc:
        with tc.tile_pool(name="persist", bufs=1) as pp, \
             tc.tile_pool(name="rawp", bufs=3) as rawp, \
             tc.tile_pool(name="swpp", bufs=4) as swpp, \
             tc.tile_pool(name="ptp", bufs=3) as ptp, \
             tc.tile_pool(name="stagep", bufs=4) as stagep, \
             tc.tile_pool(name="rrp", bufs=4) as rrp, \
             tc.tile_pool(name="ysbp", bufs=2) as ysbp, \
             tc.tile_pool(name="xqp", bufs=2) as xqp, \
             tc.tile_pool(name="psA", bufs=3, space="PSUM") as psA, \
             tc.tile_pool(name="psO", bufs=2, space="PSUM") as psO:

            # ---- persistent SBUF tensors ----
            wqk_sb = pp.tile([128, 8 * 512], B16, tag="wqk")
            wv_sb = pp.tile([128, 8 * 256], B16, tag="wv")
            wo_sb = pp.tile([128, 2 * 1024], B16, tag="wo")
            cos_sb = pp.tile([128, T], B16, tag="cos")
            sin_sb = pp.tile([128, T], B16, tag="sin")
            maskT_sb = pp.tile([128, 128], B16, tag="maskT")
            id_sb = pp.tile([128, 128], B16, tag="id")
            v_sb = pp.tile([128, 16 * HPC * 65], B16, tag="v")
            qk_rot = [pp.tile([128, T], B16, tag=f"rot{i}", name=f"rot{i}") for i in range(4)]
            aou = [pp.tile([128, T], B16, tag=f"aou{i}", name=f"aou{i}") for i in range(2)]

            # initial loads: interleave per k-chunk so accumulation chains
            # can begin as chunks arrive
            # batched loads: ONE DMA per tensor (sync-queue issue rate is
            # ~0.65us/instruction, so chunked loads serialize on issue)
            xq_tiles = {}
            xq_tiles[0] = xqp.tile([128, 8 * QT], B16, tag="xq", name="xq0")
            nc.sync.dma_start(wqk_sb.rearrange("p (k c) -> p k c", k=8),
                              wqk.rearrange("(k p) c -> p k c", p=128))
            nc.sync.dma_start(wv_sb.rearrange("p (k c) -> p k c", k=8),
                              wv.rearrange("(k p) c -> p k c", p=128))
            nc.sync.dma_start(xq_tiles[0].rearrange("p (k c) -> p k c", k=8),
                              xT[:, 0:QT].rearrange("(k p) c -> p k c", p=128))
            nc.sync.dma_start(cos_sb[:], cosR[:])
            nc.sync.dma_start(sin_sb[:], sinS[:])
            nc.sync.dma_start(maskT_sb[:], maskT[:])
            nc.sync.dma_start(id_sb[:], ident[:])
            # ones column at col 64 of every v slot
            vcols = v_sb.rearrange("p (s d) -> p s d", d=65)
            nc.vector.memset(vcols[:, :, 64:65], 1.0)
            # SWDGE warmup: boot the gpsimd DMA path before the first
            # partition_broadcast sits on a critical chain
            wrm = rrp.tile([64, 64], B16, tag="wrm")
            nc.gpsimd.partition_broadcast(wrm[:], maskT_sb[0:1, 0:64])

            # ---------- QKV projection + RoPE ----------
            def qk_chain(q, m):
                t0 = q * QT
                xq_sb = xq_tiles[q]
                ps = psA.tile([128, 1024], F32, tag="grp", name=f"qk{q}_{m}")
                for n in range(2):
                    for k in range(8):
                        nc.tensor.matmul(ps[:, n * 512:(n + 1) * 512],
                                         wqk_sb[:, k * 512 + m * 128: k * 512 + (m + 1) * 128],
                                         xq_sb[:, k * QT + n * 512: k * QT + (n + 1) * 512],
                                         start=(k == 0), stop=(k == 7))
                raw = rawp.tile([128, QT], B16, tag="raw")
                nc.scalar.copy(raw[:], ps[:])
                rot = qk_rot[m]
                cs = cos_sb[:, t0:t0 + QT]
                sn = sin_sb[:, t0:t0 + QT]
                tmp = swpp.tile([128, QT], B16, tag="tmp")
                # rotate-half via cross-base muls: out base differs, both
                # INPUTS share a base (walrus requirement), so sinS rows are
                # laid out to align with the swapped source rows
                nc.vector.tensor_mul(tmp[0:32, :], raw[32:64, :], sn[32:64, :])
                nc.vector.tensor_mul(tmp[32:64, :], raw[0:32, :], sn[0:32, :])
                nc.vector.tensor_mul(tmp[64:96, :], raw[96:128, :], sn[96:128, :])
                nc.vector.tensor_mul(tmp[96:128, :], raw[64:96, :], sn[64:96, :])
                nc.vector.tensor_mul(rot[:, t0:t0 + QT], raw[:], cs)
                nc.vector.tensor_add(rot[:, t0:t0 + QT], rot[:, t0:t0 + QT], tmp[:])

            def v_pair(q, j):
                # v for kb = q*8 + 2j, q*8 + 2j + 1
                xq_sb = xq_tiles[q]
                ps = psA.tile([128, 512], F32, tag="grp", name=f"v{q}_{j}")
                for i, mt in enumerate((2 * j, 2 * j + 1)):
                    for k in range(8):
                        nc.tensor.matmul(ps[:, i * 256:(i + 1) * 256],
                                         xq_sb[:, k * QT + mt * 128: k * QT + (mt + 1) * 128],
                                         wv_sb[:, k * 256:(k + 1) * 256],
                                         start=(k == 0), stop=(k == 7))
                vsrc = ps.rearrange("p (s h d) -> p s h d", s=2, h=4)
                base = (q * 8 + 2 * j) * 4 * 65
                vdst = v_sb[:, base:base + 2 * 4 * 65] \
                    .rearrange("p (s h e) -> p s h e", s=2, e=65)[:, :, :, 0:64]
                nc.scalar.copy(vdst, vsrc)

            # ---------- attention ----------
            def st_group(qb, h, g, st):
                h2 = h % 2
                Qt = qk_rot[0] if h < 2 else qk_rot[1]
                Kt = qk_rot[2] if h < 2 else qk_rot[3]
                r0 = 64 * h2
                for i in range(2):
                    kb = 2 * g + i
                    kt = Kt[r0:r0 + 64, kb * 128:(kb + 1) * 128]
                    if kb < 4 * qb:
                        nc.tensor.matmul(
                            st[:, i * 512:(i + 1) * 512], kt,
                            Qt[r0:r0 + 64, qb * 512:(qb + 1) * 512],
                            start=True, stop=True)
                    else:
                        dl = (kb - 4 * qb) * 128
                        if dl < 384:
                            nc.tensor.matmul(
                                st[:, i * 512 + dl + 128:(i + 1) * 512], kt,
                                Qt[r0:r0 + 64, qb * 512 + dl + 128:(qb + 1) * 512],
                                start=True, stop=True)
                        nc.tensor.matmul(
                            st[:, i * 512 + dl: i * 512 + dl + 128], kt,
                            Qt[r0:r0 + 64, qb * 512 + dl: qb * 512 + dl + 128],
                            start=True, stop=False)
                for i in range(2):
                    kb = 2 * g + i
                    if kb >= 4 * qb:
                        dl = (kb - 4 * qb) * 128
                        nc.tensor.matmul(
                            st[:, i * 512 + dl: i * 512 + dl + 128],
                            id_sb[:], maskT_sb[:],
                            start=False, stop=True)

            def pv_group(qb, h, g, pt, out_ps, live):
                for i in range(2):
                    kb = 2 * g + i
                    diag = kb >= 4 * qb
                    dl = (kb - 4 * qb) * 128 if diag else 0
                    nc.tensor.matmul(
                        out_ps[:, dl:512],
                        v_sb[:, (kb * 4 + h) * 65:(kb * 4 + h) * 65 + 65],
                        pt[:, i * 512 + dl:(i + 1) * 512],
                        start=(kb == 0), stop=(kb == live - 1))

            norm_state = {}

            def att_block(qb, hp, hpi):
                """Generator: one yield per g-step; ends after norm1."""
                live = 4 * (qb + 1)
                ng = live // 2
                out_ps = {}
                pts = {h: {} for h in hp}
                for g in range(ng + 1):
                    for h in hp:
                        if g < ng:
                            st = psA.tile([128, 1024], F32, tag="grp",
                                          name=f"st{qb}_{h}_{g}")
                            st_group(qb, h, g, st)
                            if g < 2 * qb:
                                ranges = ((0, 1024),)
                            elif g == 2 * qb:
                                ranges = ((0, 512), (640, 1024))
                            else:
                                ranges = ((256, 512), (896, 1024))
                            pt = ptp.tile([128, 1024], B16, tag="pt",
                                          name=f"pt{qb}_{h}_{g}")
                            for lo, hi in ranges:
                                nc.scalar.activation(pt[:, lo:hi], st[:, lo:hi],
                                                     AF.Exp, scale=0.125)
                            pts[h][g] = pt
                    for h in hp:
                        if g >= 1:
                            if h not in out_ps:
                                out_ps[h] = psO.tile([65, 512], F32, tag="acc",
                                                     name=f"ops{qb}_{h}")
                            pv_group(qb, h, g - 1, pts[h].pop(g - 1),
                                     out_ps[h], live)
                    if g < ng:
                        yield
                # norm part 1: stage both heads side by side; one reciprocal
                stage = stagep.tile([65, 1024], F32, tag="stage",
                                    name=f"stage{qb}_{hpi}")
                nc.vector.tensor_copy(stage[:, 0:512], out_ps[hp[0]][:])
                nc.vector.tensor_copy(stage[:, 512:1024], out_ps[hp[1]][:])
                den0 = rrp.tile([1, 1024], F32, tag="den", name=f"den{qb}_{hpi}")
                nc.sync.dma_start(den0[:], stage[64:65, :])
                rr = rrp.tile([1, 1024], F32, tag="rr", name=f"rr{qb}_{hpi}")
                with nc.allow_low_precision(reason="softmax denominators"):
                    nc.vector.reciprocal_approx_fast(rr[:], den0[:])
                rrb = rrp.tile([64, 1024], F32, tag="rrb", name=f"rrb{qb}_{hpi}")
                nc.gpsimd.partition_broadcast(rrb[:], rr[0:1, :])
                if debug and qb == 0 and hpi == 0:
                    nc.sync.dma_start(d_stage[:], stage[:])
                    nc.sync.dma_start(d_rr[:], rr[:])
                    nc.sync.dma_start(d_rrr[:], rrb[0:1, :])
                norm_state[(qb, hpi)] = (stage, rrb)
                yield

            def norm2(qb, hpi):
                stage, rrb = norm_state.pop((qb, hpi))
                AO = aou[hpi]
                nc.vector.tensor_mul(AO[0:64, qb * 512:(qb + 1) * 512],
                                     stage[0:64, 0:512], rrb[:, 0:512])
                nc.vector.tensor_mul(AO[64:128, qb * 512:(qb + 1) * 512],
                                     stage[0:64, 512:1024], rrb[:, 512:1024])

            def outproj_qt(qt):
                yp = psA.tile([128, 1024], F32, tag="grp", name=f"yp{qt}")
                for nh in range(2):
                    nc.tensor.matmul(yp[:, nh * 512:(nh + 1) * 512],
                                     aou[0][:, qt * 128:(qt + 1) * 128],
                                     wo_sb[:, nh * 512:(nh + 1) * 512],
                                     start=True, stop=False)
                    nc.tensor.matmul(yp[:, nh * 512:(nh + 1) * 512],
                                     aou[1][:, qt * 128:(qt + 1) * 128],
                                     wo_sb[:, 1024 + nh * 512:1024 + (nh + 1) * 512],
                                     start=False, stop=True)
                ysb = ysbp.tile([128, 1024], B16, tag="y", name=f"ysb{qt}")
                nc.vector.tensor_copy(ysb[:], yp[:])
                nc.sync.dma_start(out[qt * 128:(qt + 1) * 128, :], ysb[:])

            # ================= schedule =================
            # --- phase 1: QKV half 0, attention qb=1 hp0 interleaved
            gen0 = att_block(1, (0, 1), 0)
            qk_chain(0, 0)
            qk_chain(0, 2)
            for j in range(4):
                v_pair(0, j)
                next(gen0, None)
            qk_chain(0, 1)
            # prefetch x half 1 + deferred weights
            xq_tiles[1] = xqp.tile([128, 8 * QT], B16, tag="xq", name="xq1")
            for k in range(8):
                nc.sync.dma_start(xq_tiles[1][:, k * QT:(k + 1) * QT],
                                  xT[k * 128:(k + 1) * 128, QT:T])
            for k in range(2):
                nc.sync.dma_start(wo_sb[:, k * 1024:(k + 1) * 1024], wo[k * 128:(k + 1) * 128, :])
            for _ in gen0:
                pass
            qk_chain(0, 3)

            # --- phase 2: QKV half 1, attention qb=1 hp1 interleaved
            gen1 = att_block(1, (2, 3), 1)
            qk_chain(1, 0)
            next(gen1, None)
            qk_chain(1, 2)
            next(gen1, None)
            for j in range(4):
                v_pair(1, j)
                next(gen1, None)
            qk_chain(1, 1)
            for _ in gen1:
                pass
            qk_chain(1, 3)

            # --- remaining query blocks: 3, 2, 0; outproj/norm2 one qb late
            def run_block(qb, fillers):
                # delay fillers a couple of g-steps so the bc matmul (which
                # waits on the previous block's reciprocal chain) never heads
                # the PE queue while early st-groups could run
                fillers = list(fillers)
                total = 2 * (2 * (qb + 1) + 1)
                skip = max(0, min(2, total - len(fillers)))
                step = 0
                for hpi, hp in enumerate(((0, 1), (2, 3))):
                    for _ in att_block(qb, hp, hpi):
                        if fillers and step >= skip:
                            fillers.pop(0)()
                        step += 1
                for f in fillers:
                    f()

            def mk_fillers(pqb):
                fs = [lambda: norm2(pqb, 0), lambda: norm2(pqb, 1)]
                fs += [(lambda qt=qt: outproj_qt(qt)) for qt in range(4 * pqb, 4 * pqb + 4)]
                return fs

            run_block(3, mk_fillers(1))
            run_block(2, mk_fillers(3))
            # qb0 + tail, hand-scheduled: norm2(0,0) lands inside the hp1
            # block; outproj(0) is split so its aou[0] halves run while the
            # hp1 normalization chain completes
            f2 = mk_fillers(2)
            for _ in att_block(0, (0, 1), 0):
                if f2:
                    f2.pop(0)()
            gtail = att_block(0, (2, 3), 1)
            next(gtail, None)
            if f2:
                f2.pop(0)()
            next(gtail, None)
            norm2(0, 0)
            for _ in gtail:
                if f2:
                    f2.pop(0)()
            for f in f2:
                f()
            norm2(0, 1)
            yps = {}
            for qt in range(0, 3):
                yp = psA.tile([128, 1024], F32, tag="grp", name=f"yp{qt}")
                for nh in range(2):
                    nc.tensor.matmul(yp[:, nh * 512:(nh + 1) * 512],
                                     aou[0][:, qt * 128:(qt + 1) * 128],
                                     wo_sb[:, nh * 512:(nh + 1) * 512],
                                     start=True, stop=False)
                yps[qt] = yp
            for qt in range(0, 3):
                yp = yps[qt]
                for nh in range(2):
                    nc.tensor.matmul(yp[:, nh * 512:(nh + 1) * 512],
                                     aou[1][:, qt * 128:(qt + 1) * 128],
                                     wo_sb[:, 1024 + nh * 512:1024 + (nh + 1) * 512],
                                     start=False, stop=True)
                ysb = ysbp.tile([128, 1024], B16, tag="y", name=f"ysb{qt}")
                nc.vector.tensor_copy(ysb[:], yp[:])
                nc.sync.dma_start(out[qt * 128:(qt + 1) * 128, :], ysb[:])
            outproj_qt(3)
            if debug:
                for i in range(4):
                    nc.sync.dma_start(d_rot[i][:], qk_rot[i][:])
                nc.sync.dma_start(d_v[:], v_sb[:])
                for i in range(2):
                    nc.sync.dma_start(d_aou[i][:], aou[i][:])

    nc.compile()
    return nc


def _core_inputs(x, cos, sin, W_qkv, W_out, core):
    b = core // 4
    hg = core % 4
    heads = list(range(4 * hg, 4 * hg + 4))

    xT = np.ascontiguousarray(x[b].T).astype(BF)
    qrows = np.concatenate([W_qkv[h * 64:(h + 1) * 64] for h in heads], 0)
    krows = np.concatenate([W_qkv[C + h * 64: C + (h + 1) * 64] for h in heads], 0)
    wqk = np.ascontiguousarray(np.concatenate([qrows, krows], 0).T).astype(BF)
    vrows = np.concatenate([W_qkv[2 * C + h * 64: 2 * C + (h + 1) * 64] for h in heads], 0)
    wv = np.ascontiguousarray(vrows.T).astype(BF)
    cols = np.concatenate([np.arange(h * 64, (h + 1) * 64) for h in heads])
    wo = np.ascontiguousarray(W_out[:, cols].T).astype(BF)

    cT = np.ascontiguousarray(cos.T)      # (32, T)
    sT = np.ascontiguousarray(sin.T)
    cosR = np.tile(cT, (4, 1)).astype(BF)
    # row block b holds the sin factor for the SWAPPED source living at
    # block b: tmp[0:32]=raw[32:64]*(-s) reads sin rows 32:64, etc.
    sinS = np.concatenate([sT, -sT, sT, -sT], 0).astype(BF)

    p = np.arange(128)[:, None]
    j = np.arange(128)[None, :]
    maskT = np.where(p <= j, 0.0, NEG).astype(BF)

    return {
        "xT": xT, "wqk": wqk, "wv": wv, "wo": wo,
        "cosR": cosR, "sinS": sinS,
        "maskT": np.ascontiguousarray(maskT),
        "ident": np.eye(128).astype(BF),
    }


def kernel(x, cos, sin, mask, W_qkv, W_out):
    from concourse import bass_utils

    x = np.asarray(x, dtype=np.float32)
    cos = np.asarray(cos, dtype=np.float32)
    sin = np.asarray(sin, dtype=np.float32)
    W_qkv = np.asarray(W_qkv, dtype=np.float32)
    W_out = np.asarray(W_out, dtype=np.float32)

    if "nc" not in _cache:
        _cache["nc"] = _build()
    nc = _cache["nc"]

    in_maps = [_core_inputs(x, cos, sin, W_qkv, W_out, c) for c in range(NCORES)]
    res = bass_utils.run_bass_kernel_spmd(nc, in_maps, core_ids=list(range(NCORES)))

    y = np.zeros((B, T, C), dtype=np.float32)
    for c in range(NCORES):
        y[c // 4] += res.results[c]["out"].astype(np.float32)
    return y


# revision 34
# speedup vs baseline: 1.1740x; 1.0182x over previous
"""Fused multi-head causal attention (RoPE) for Trainium2, 8-core SPMD.

Sharding: data-parallel over batch (B=2) x tensor-parallel over heads
(16 heads -> 4 per core, Megatron-style column/row split of the qkv/out
projections). Each core computes a partial (T, C) output; the host sums
the 4 partials per batch element.

v2 schedule (vs v1): attention for qb=1 (queries 512:1023, keys 0:1023
-- fully computable from the first token half) is interleaved into the
QKV projection of both halves so the Scalar engine's exp work starts
~40us earlier; remaining query blocks run in order 3,2,0 so the short
qb=0 block forms the tail.  Causal-mask adds stream a single [128,128]
tril tile (128 cols/diag-block instead of 512-dl).  Softmax
normalization is fused per head-pair: one reciprocal over [1,1024], one
K=1 broadcast matmul (fp32->f32r bitcast, no copy), and the normalize
multiply writes straight into the aou tiles (no intermediate DMA).
Output is stored bf16 (host accumulates in fp32).
"""

import sys
import numpy as np

if '/opt/trn_rl_repo' not in sys.path:
    sys.path.insert(0, '/opt/trn_rl_repo')

import ml_dtypes

B, T, C, H, D = 2, 2048, 1024, 16, 64
HPC = 4            # heads per core
NCORES = 8
NEG = -1.0e9
BF = ml_dtypes.bfloat16

_cache = {}


def _build(debug=False):
    import concourse.mybir as mybir
    from concourse import bacc
    import concourse.tile as tile

    F32 = mybir.dt.float32
    FR = mybir.dt.float32r
    B16 = mybir.dt.bfloat16
    AF = mybir.ActivationFunctionType

    nc = bacc.Bacc("TRN2", debug=False, enable_asserts=True)
    xT = nc.dram_tensor("xT", [C, T], B16, kind="ExternalInput").ap()
    wqk = nc.dram_tensor("wqk", [C, 512], B16, kind="ExternalInput").ap()
    wv = nc.dram_tensor("wv", [C, 256], B16, kind="ExternalInput").ap()
    wo = nc.dram_tensor("wo", [256, 1024], B16, kind="ExternalInput").ap()
    cosR = nc.dram_tensor("cosR", [128, T], B16, kind="ExternalInput").ap()
    sinS = nc.dram_tensor("sinS", [128, T], B16, kind="ExternalInput").ap()
    maskT = nc.dram_tensor("maskT", [128, 128], B16, kind="ExternalInput").ap()
    ident = nc.dram_tensor("ident", [128, 128], B16, kind="ExternalInput").ap()
    out = nc.dram_tensor("out", [T, C], B16, kind="ExternalOutput").ap()
    if debug:
        d_rot = [nc.dram_tensor(f"d_rot{i}", [128, T], B16, kind="ExternalOutput").ap()
                 for i in range(4)]
        d_v = nc.dram_tensor("d_v", [128, 16 * HPC * 65], B16, kind="ExternalOutput").ap()
        d_aou = [nc.dram_tensor(f"d_aou{i}", [128, T], B16, kind="ExternalOutput").ap()
                 for i in range(2)]
        d_stage = nc.dram_tensor("d_stage", [65, 1024], F32, kind="ExternalOutput").ap()
        d_rr = nc.dram_tensor("d_rr", [1, 1024], F32, kind="ExternalOutput").ap()
        d_rrr = nc.dram_tensor("d_rrr", [1, 1024], F32, kind="ExternalOutput").ap()

    NQ = 2            # token halves for streaming x
    QT = T // NQ      # 1024 tokens per half

    with tile.TileContext(nc) as tc:
        with tc.tile_pool(name="persist", bufs=1) as pp, \
             tc.tile_pool(name="rawp", bufs=3) as rawp, \
             tc.tile_pool(name="swpp", bufs=4) as swpp, \
             tc.tile_pool(name="ptp", bufs=3) as ptp, \
             tc.tile_pool(name="stagep", bufs=4) as stagep, \
             tc.tile_pool(name="rrp", bufs=4) as rrp, \
             tc.tile_pool(name="ysbp", bufs=2) as ysbp, \
             tc.tile_pool(name="xqp", bufs=2) as xqp, \
             tc.tile_pool(name="psA", bufs=3, space="PSUM") as psA, \
             tc.tile_pool(name="psO", bufs=2, space="PSUM") as psO:

            # ---- persistent SBUF tensors ----
            wqk_sb = pp.tile([128, 8 * 512], B16, tag="wqk")
            wv_sb = pp.tile([128, 8 * 256], B16, tag="wv")
            wo_sb = pp.tile([128, 2 * 1024], B16, tag="wo")
            cos_sb = pp.tile([128, T], B16, tag="cos")
            sin_sb = pp.tile([128, T], B16, tag="sin")
            maskT_sb = pp.tile([128, 128], B16, tag="maskT")
            id_sb = pp.tile([128, 128], B16, tag="id")
            v_sb = pp.tile([128, 16 * HPC * 65], B16, tag="v")
            qk_rot = [pp.tile([128, T], B16, tag=f"rot{i}", name=f"rot{i}") for i in range(4)]
            aou = [pp.tile([128, T], B16, tag=f"aou{i}", name=f"aou{i}") for i in range(2)]

            # initial loads: interleave per k-chunk so accumulation chains
            # can begin as chunks arrive
            # batched loads, few DMA instructions (sync-queue issue rate is
            # ~0.65us/instruction), ordered so the first qk chains can start
            # after the first wqk/x half arrives
            xq_tiles = {}
            xq_tiles[0] = xqp.tile([128, 8 * QT], B16, tag="xq", name="xq0")
            xq_tiles[1] = xqp.tile([128, 8 * QT], B16, tag="xq", name="xq1")
            for kh in range(2):
                k0 = kh * 4
                nc.sync.dma_start(
                    wqk_sb[:, k0 * 512:(k0 + 4) * 512].rearrange("p (k c) -> p k c", k=4),
                    wqk[k0 * 128:(k0 + 4) * 128, :].rearrange("(k p) c -> p k c", p=128))
                nc.sync.dma_start(
                    xq_tiles[0][:, k0 * QT:(k0 + 4) * QT].rearrange("p (k c) -> p k c", k=4),
                    xT[k0 * 128:(k0 + 4) * 128, 0:QT].rearrange("(k p) c -> p k c", p=128))
            nc.sync.dma_start(wv_sb.rearrange("p (k c) -> p k c", k=8),
                              wv.rearrange("(k p) c -> p k c", p=128))
            nc.sync.dma_start(cos_sb[:], cosR[:])
            nc.sync.dma_start(sin_sb[:], sinS[:])
            nc.sync.dma_start(maskT_sb[:], maskT[:])
            nc.sync.dma_start(id_sb[:], ident[:])
            nc.sync.dma_start(xq_tiles[1].rearrange("p (k c) -> p k c", k=8),
                              xT[:, QT:T].rearrange("(k p) c -> p k c", p=128))
            nc.sync.dma_start(wo_sb.rearrange("p (k c) -> p k c", k=2),
                              wo.rearrange("(k p) c -> p k c", p=128))
            # ones column at col 64 of every v slot
            vcols = v_sb.rearrange("p (s d) -> p s d", d=65)
            nc.vector.memset(vcols[:, :, 64:65], 1.0)
            # SWDGE warmup: boot the gpsimd DMA path before the first
            # partition_broadcast sits on a critical chain
            wrm = rrp.tile([64, 64], B16, tag="wrm")
            nc.gpsimd.partition_broadcast(wrm[:], maskT_sb[0:1, 0:64])

            # ---------- QKV projection + RoPE ----------
            def qk_chain(q, m):
                t0 = q * QT
                xq_sb = xq_tiles[q]
                ps = psA.tile([128, 1024], F32, tag="grp", name=f"qk{q}_{m}")
                # k-halves outer so accumulation starts after the first
                # wqk/x half-load lands
                for kh in range(2):
                    for n in range(2):
                        for k in range(kh * 4, kh * 4 + 4):
                            nc.tensor.matmul(ps[:, n * 512:(n + 1) * 512],
                                             wqk_sb[:, k * 512 + m * 128: k * 512 + (m + 1) * 128],
                                             xq_sb[:, k * QT + n * 512: k * QT + (n + 1) * 512],
                                             start=(k == 0), stop=(k == 7))
                raw = rawp.tile([128, QT], B16, tag="raw")
                nc.scalar.copy(raw[:], ps[:])
                rot = qk_rot[m]
                cs = cos_sb[:, t0:t0 + QT]
                sn = sin_sb[:, t0:t0 + QT]
                tmp = swpp.tile([128, QT], B16, tag="tmp")
                # rotate-half via cross-base muls: out base differs, both
                # INPUTS share a base (walrus requirement), so sinS rows are
                # laid out to align with the swapped source rows
                nc.vector.tensor_mul(tmp[0:32, :], raw[32:64, :], sn[32:64, :])
                nc.vector.tensor_mul(tmp[32:64, :], raw[0:32, :], sn[0:32, :])
                nc.vector.tensor_mul(tmp[64:96, :], raw[96:128, :], sn[96:128, :])
                nc.vector.tensor_mul(tmp[96:128, :], raw[64:96, :], sn[64:96, :])
                nc.vector.tensor_mul(rot[:, t0:t0 + QT], raw[:], cs)
                nc.vector.tensor_add(rot[:, t0:t0 + QT], rot[:, t0:t0 + QT], tmp[:])

            def v_pair(q, j):
                # v for kb = q*8 + 2j, q*8 + 2j + 1
                xq_sb = xq_tiles[q]
                ps = psA.tile([128, 512], F32, tag="grp", name=f"v{q}_{j}")
                for i, mt in enumerate((2 * j, 2 * j + 1)):
                    for k in range(8):
                        nc.tensor.matmul(ps[:, i * 256:(i + 1) * 256],
                                         xq_sb[:, k * QT + mt * 128: k * QT + (mt + 1) * 128],
                                         wv_sb[:, k * 256:(k + 1) * 256],
                                         start=(k == 0), stop=(k == 7))
                vsrc = ps.rearrange("p (s h d) -> p s h d", s=2, h=4)
                base = (q * 8 + 2 * j) * 4 * 65
                vdst = v_sb[:, base:base + 2 * 4 * 65] \
                    .rearrange("p (s h e) -> p s h e", s=2, e=65)[:, :, :, 0:64]
                nc.scalar.copy(vdst, vsrc)

            # ---------- attention ----------
            def st_group(qb, h, g, st):
                h2 = h % 2
                Qt = qk_rot[0] if h < 2 else qk_rot[1]
                Kt = qk_rot[2] if h < 2 else qk_rot[3]
                r0 = 64 * h2
                for i in range(2):
                    kb = 2 * g + i
                    kt = Kt[r0:r0 + 64, kb * 128:(kb + 1) * 128]
                    if kb < 4 * qb:
                        nc.tensor.matmul(
                            st[:, i * 512:(i + 1) * 512], kt,
                            Qt[r0:r0 + 64, qb * 512:(qb + 1) * 512],
                            start=True, stop=True)
                    else:
                        dl = (kb - 4 * qb) * 128
                        if dl < 384:
                            nc.tensor.matmul(
                                st[:, i * 512 + dl + 128:(i + 1) * 512], kt,
                                Qt[r0:r0 + 64, qb * 512 + dl + 128:(qb + 1) * 512],
                                start=True, stop=True)
                        nc.tensor.matmul(
                            st[:, i * 512 + dl: i * 512 + dl + 128], kt,
                            Qt[r0:r0 + 64, qb * 512 + dl: qb * 512 + dl + 128],
                            start=True, stop=False)
                for i in range(2):
                    kb = 2 * g + i
                    if kb >= 4 * qb:
                        dl = (kb - 4 * qb) * 128
                        nc.tensor.matmul(
                            st[:, i * 512 + dl: i * 512 + dl + 128],
                            id_sb[:], maskT_sb[:],
                            start=False, stop=True)

            def pv_group(qb, h, g, pt, out_ps, live):
                for i in range(2):
                    kb = 2 * g + i
                    diag = kb >= 4 * qb
                    dl = (kb - 4 * qb) * 128 if diag else 0
                    nc.tensor.matmul(
                        out_ps[:, dl:512],
                        v_sb[:, (kb * 4 + h) * 65:(kb * 4 + h) * 65 + 65],
                        pt[:, i * 512 + dl:(i + 1) * 512],
                        start=(kb == 0), stop=(kb == live - 1))

            norm_state = {}

            def att_block(qb, hp, hpi):
                """Generator: one yield per g-step; ends after norm1."""
                live = 4 * (qb + 1)
                ng = live // 2
                out_ps = {}
                pts = {h: {} for h in hp}
                for g in range(ng + 1):
                    for h in hp:
                        if g < ng:
                            st = psA.tile([128, 1024], F32, tag="grp",
                                          name=f"st{qb}_{h}_{g}")
                            st_group(qb, h, g, st)
                            if g < 2 * qb:
                                ranges = ((0, 1024),)
                            elif g == 2 * qb:
                                ranges = ((0, 512), (640, 1024))
                            else:
                                ranges = ((256, 512), (896, 1024))
                            pt = ptp.tile([128, 1024], B16, tag="pt",
                                          name=f"pt{qb}_{h}_{g}")
                            for lo, hi in ranges:
                                nc.scalar.activation(pt[:, lo:hi], st[:, lo:hi],
                                                     AF.Exp, scale=0.125)
                            pts[h][g] = pt
                    for h in hp:
                        if g >= 1:
                            if h not in out_ps:
                                out_ps[h] = psO.tile([65, 512], F32, tag="acc",
                                                     name=f"ops{qb}_{h}")
                            pv_group(qb, h, g - 1, pts[h].pop(g - 1),
                                     out_ps[h], live)
                    if g < ng:
                        yield
                # norm part 1: stage both heads side by side; one reciprocal
                stage = stagep.tile([65, 1024], F32, tag="stage",
                                    name=f"stage{qb}_{hpi}")
                nc.vector.tensor_copy(stage[:, 0:512], out_ps[hp[0]][:])
                nc.vector.tensor_copy(stage[:, 512:1024], out_ps[hp[1]][:])
                den0 = rrp.tile([1, 1024], F32, tag="den", name=f"den{qb}_{hpi}")
                nc.sync.dma_start(den0[:], stage[64:65, :])
                rr = rrp.tile([1, 1024], F32, tag="rr", name=f"rr{qb}_{hpi}")
                with nc.allow_low_precision(reason="softmax denominators"):
                    nc.vector.reciprocal_approx_fast(rr[:], den0[:])
                rrb = rrp.tile([64, 1024], F32, tag="rrb", name=f"rrb{qb}_{hpi}")
                nc.gpsimd.partition_broadcast(rrb[:], rr[0:1, :])
                if debug and qb == 0 and hpi == 0:
                    nc.sync.dma_start(d_stage[:], stage[:])
                    nc.sync.dma_start(d_rr[:], rr[:])
                    nc.sync.dma_start(d_rrr[:], rrb[0:1, :])
                norm_state[(qb, hpi)] = (stage, rrb)
                yield

            def norm2(qb, hpi):
                stage, rrb = norm_state.pop((qb, hpi))
                AO = aou[hpi]
                nc.vector.tensor_mul(AO[0:64, qb * 512:(qb + 1) * 512],
                                     stage[0:64, 0:512], rrb[:, 0:512])
                nc.vector.tensor_mul(AO[64:128, qb * 512:(qb + 1) * 512],
                                     stage[0:64, 512:1024], rrb[:, 512:1024])

            def outproj_qt(qt):
                yp = psA.tile([128, 1024], F32, tag="grp", name=f"yp{qt}")
                for nh in range(2):
                    nc.tensor.matmul(yp[:, nh * 512:(nh + 1) * 512],
                                     aou[0][:, qt * 128:(qt + 1) * 128],
                                     wo_sb[:, nh * 512:(nh + 1) * 512],
                                     start=True, stop=False)
                    nc.tensor.matmul(yp[:, nh * 512:(nh + 1) * 512],
                                     aou[1][:, qt * 128:(qt + 1) * 128],
                                     wo_sb[:, 1024 + nh * 512:1024 + (nh + 1) * 512],
                                     start=False, stop=True)
                ysb = ysbp.tile([128, 1024], B16, tag="y", name=f"ysb{qt}")
                nc.vector.tensor_copy(ysb[:], yp[:])
                nc.sync.dma_start(out[qt * 128:(qt + 1) * 128, :], ysb[:])

            # ================= schedule =================
            # --- phase 1: QKV half 0, attention qb=1 hp0 interleaved
            gen0 = att_block(1, (0, 1), 0)
            qk_chain(0, 0)
            qk_chain(0, 2)
            for j in range(4):
                v_pair(0, j)
                next(gen0, None)
            qk_chain(0, 1)
            for _ in gen0:
                pass
            qk_chain(0, 3)

            # --- phase 2: QKV half 1, attention qb=1 hp1 interleaved.
            # m1/m3 before the v-pairs so all rope chains (DVE) drain early
            # and qb3 never waits on them.
            gen1 = att_block(1, (2, 3), 1)
            qk_chain(1, 0)
            next(gen1, None)
            qk_chain(1, 2)
            next(gen1, None)
            qk_chain(1, 1)
            next(gen1, None)
            qk_chain(1, 3)
            next(gen1, None)
            v_pair(1, 0)
            for _ in gen1:
                pass
            for j in range(1, 4):
                v_pair(1, j)

            # --- remaining query blocks: 3, 2, 0; outproj/norm2 one qb late
            def run_block(qb, fillers):
                # delay fillers a couple of g-steps so the bc matmul (which
                # waits on the previous block's reciprocal chain) never heads
                # the PE queue while early st-groups could run
                fillers = list(fillers)
                total = 2 * (2 * (qb + 1) + 1)
                skip = max(0, min(2, total - len(fillers)))
                step = 0
                for hpi, hp in enumerate(((0, 1), (2, 3))):
                    for _ in att_block(qb, hp, hpi):
                        if fillers and step >= skip:
                            fillers.pop(0)()
                        step += 1
                for f in fillers:
                    f()

            def mk_fillers(pqb):
                fs = [lambda: norm2(pqb, 0), lambda: norm2(pqb, 1)]
                fs += [(lambda qt=qt: outproj_qt(qt)) for qt in range(4 * pqb, 4 * pqb + 4)]
                return fs

            run_block(3, mk_fillers(1))
            run_block(2, mk_fillers(3))
            # qb0 + tail, hand-scheduled: norm2(0,0) lands inside the hp1
            # block; outproj(0) is split so its aou[0] halves run while the
            # hp1 normalization chain completes
            f2 = mk_fillers(2)
            for _ in att_block(0, (0, 1), 0):
                if f2:
                    f2.pop(0)()
            gtail = att_block(0, (2, 3), 1)
            next(gtail, None)
            if f2:
                f2.pop(0)()
            next(gtail, None)
            norm2(0, 0)
            for _ in gtail:
                if f2:
                    f2.pop(0)()
            for f in f2:
                f()
            norm2(0, 1)
            yps = {}
            for qt in range(0, 3):
                yp = psA.tile([128, 1024], F32, tag="grp", name=f"yp{qt}")
                for nh in range(2):
                    nc.tensor.matmul(yp[:, nh * 512:(nh + 1) * 512],
                                     aou[0][:, qt * 128:(qt + 1) * 128],
                                     wo_sb[:, nh * 512:(nh + 1) * 512],
                                     start=True, stop=False)
                yps[qt] = yp
            for qt in range(0, 3):
                yp = yps[qt]
                for nh in range(2):
                    nc.tensor.matmul(yp[:, nh * 512:(nh + 1) * 512],
                                     aou[1][:, qt * 128:(qt + 1) * 128],
                                     wo_sb[:, 1024 + nh * 512:1024 + (nh + 1) * 512],
                                     start=False, stop=True)
                ysb = ysbp.tile([128, 1024], B16, tag="y", name=f"ysb{qt}")
                nc.vector.tensor_copy(ysb[:], yp[:])
                nc.sync.dma_start(out[qt * 128:(qt + 1) * 128, :], ysb[:])
            outproj_qt(3)
            if debug:
                for i in range(4):
                    nc.sync.dma_start(d_rot[i][:], qk_rot[i][:])
                nc.sync.dma_start(d_v[:], v_sb[:])
                for i in range(2):
                    nc.sync.dma_start(d_aou[i][:], aou[i][:])

    nc.compile()
    return nc


def _core_inputs(x, cos, sin, W_qkv, W_out, core):
    b = core // 4
    hg = core % 4
    heads = list(range(4 * hg, 4 * hg + 4))

    xT = np.ascontiguousarray(x[b].T).astype(BF)
    qrows = np.concatenate([W_qkv[h * 64:(h + 1) * 64] for h in heads], 0)
    krows = np.concatenate([W_qkv[C + h * 64: C + (h + 1) * 64] for h in heads], 0)
    wqk = np.ascontiguousarray(np.concatenate([qrows, krows], 0).T).astype(BF)
    vrows = np.concatenate([W_qkv[2 * C + h * 64: 2 * C + (h + 1) * 64] for h in heads], 0)
    wv = np.ascontiguousarray(vrows.T).astype(BF)
    cols = np.concatenate([np.arange(h * 64, (h + 1) * 64) for h in heads])
    wo = np.ascontiguousarray(W_out[:, cols].T).astype(BF)

    cT = np.ascontiguousarray(cos.T)      # (32, T)
    sT = np.ascontiguousarray(sin.T)
    cosR = np.tile(cT, (4, 1)).astype(BF)
    # row block b holds the sin factor for the SWAPPED source living at
    # block b: tmp[0:32]=raw[32:64]*(-s) reads sin rows 32:64, etc.
    sinS = np.concatenate([sT, -sT, sT, -sT], 0).astype(BF)

    p = np.arange(128)[:, None]
    j = np.arange(128)[None, :]
    maskT = np.where(p <= j, 0.0, NEG).astype(BF)

    return {
        "xT": xT, "wqk": wqk, "wv": wv, "wo": wo,
        "cosR": cosR, "sinS": sinS,
        "maskT": np.ascontiguousarray(maskT),
        "ident": np.eye(128).astype(BF),
    }


def kernel(x, cos, sin, mask, W_qkv, W_out):
    from concourse import bass_utils

    x = np.asarray(x, dtype=np.float32)
    cos = np.asarray(cos, dtype=np.float32)
    sin = np.asarray(sin, dtype=np.float32)
    W_qkv = np.asarray(W_qkv, dtype=np.float32)
    W_out = np.asarray(W_out, dtype=np.float32)

    if "nc" not in _cache:
        _cache["nc"] = _build()
    nc = _cache["nc"]

    in_maps = [_core_inputs(x, cos, sin, W_qkv, W_out, c) for c in range(NCORES)]
    res = bass_utils.run_bass_kernel_spmd(nc, in_maps, core_ids=list(range(NCORES)))

    y = np.zeros((B, T, C), dtype=np.float32)
    for c in range(NCORES):
        y[c // 4] += res.results[c]["out"].astype(np.float32)
    return y


# revision 37
# speedup vs baseline: 1.2640x; 1.0767x over previous
"""Fused multi-head causal attention (RoPE) for Trainium2, 8-core SPMD.

Sharding: data-parallel over batch (B=2) x tensor-parallel over heads
(16 heads -> 4 per core, Megatron-style column/row split of the qkv/out
projections). Each core computes a partial (T, C) output; the host sums
the 4 partials per batch element.

v2 schedule (vs v1): attention for qb=1 (queries 512:1023, keys 0:1023
-- fully computable from the first token half) is interleaved into the
QKV projection of both halves so the Scalar engine's exp work starts
~40us earlier; remaining query blocks run in order 3,2,0 so the short
qb=0 block forms the tail.  Causal-mask adds stream a single [128,128]
tril tile (128 cols/diag-block instead of 512-dl).  Softmax
normalization is fused per head-pair: one reciprocal over [1,1024], one
K=1 broadcast matmul (fp32->f32r bitcast, no copy), and the normalize
multiply writes straight into the aou tiles (no intermediate DMA).
Output is stored bf16 (host accumulates in fp32).
"""

import sys
import numpy as np

if '/opt/trn_rl_repo' not in sys.path:
    sys.path.insert(0, '/opt/trn_rl_repo')

import ml_dtypes

B, T, C, H, D = 2, 2048, 1024, 16, 64
HPC = 4            # heads per core
NCORES = 8
NEG = -1.0e9
BF = ml_dtypes.bfloat16

_cache = {}


def _build(debug=False):
    import concourse.mybir as mybir
    from concourse import bacc
    import concourse.tile as tile

    F32 = mybir.dt.float32
    FR = mybir.dt.float32r
    B16 = mybir.dt.bfloat16
    AF = mybir.ActivationFunctionType

    nc = bacc.Bacc("TRN2", debug=False, enable_asserts=True)
    xT = nc.dram_tensor("xT", [C, T], B16, kind="ExternalInput").ap()
    wqk = nc.dram_tensor("wqk", [C, 512], B16, kind="ExternalInput").ap()
    wv = nc.dram_tensor("wv", [C, 256], B16, kind="ExternalInput").ap()
    wo = nc.dram_tensor("wo", [256, 1024], B16, kind="ExternalInput").ap()
    cosR = nc.dram_tensor("cosR", [128, T], B16, kind="ExternalInput").ap()
    sinS = nc.dram_tensor("sinS", [128, T], B16, kind="ExternalInput").ap()
    maskT = nc.dram_tensor("maskT", [128, 128], B16, kind="ExternalInput").ap()
    ident = nc.dram_tensor("ident", [128, 128], B16, kind="ExternalInput").ap()
    out = nc.dram_tensor("out", [T, C], B16, kind="ExternalOutput").ap()
    if debug:
        d_rot = [nc.dram_tensor(f"d_rot{i}", [128, T], B16, kind="ExternalOutput").ap()
                 for i in range(4)]
        d_v = nc.dram_tensor("d_v", [128, 16 * HPC * 65], B16, kind="ExternalOutput").ap()
        d_aou = [nc.dram_tensor(f"d_aou{i}", [128, T], B16, kind="ExternalOutput").ap()
                 for i in range(2)]
        d_stage = nc.dram_tensor("d_stage", [65, 1024], F32, kind="ExternalOutput").ap()
        d_rr = nc.dram_tensor("d_rr", [1, 1024], F32, kind="ExternalOutput").ap()
        d_rrr = nc.dram_tensor("d_rrr", [1, 1024], F32, kind="ExternalOutput").ap()

    NQ = 2            # token halves for streaming x
    QT = T // NQ      # 1024 tokens per half

    with tile.TileContext(nc) as tc:
        with tc.tile_pool(name="persist", bufs=1) as pp, \
             tc.tile_pool(name="rawp", bufs=3) as rawp, \
             tc.tile_pool(name="swpp", bufs=4) as swpp, \
             tc.tile_pool(name="ptp", bufs=3) as ptp, \
             tc.tile_pool(name="stagep", bufs=4) as stagep, \
             tc.tile_pool(name="rrp", bufs=4) as rrp, \
             tc.tile_pool(name="ysbp", bufs=2) as ysbp, \
             tc.tile_pool(name="xqp", bufs=2) as xqp, \
             tc.tile_pool(name="psA", bufs=3, space="PSUM") as psA, \
             tc.tile_pool(name="psO", bufs=2, space="PSUM") as psO:

            # ---- persistent SBUF tensors ----
            wqk_sb = pp.tile([128, 8 * 512], B16, tag="wqk")
            wv_sb = pp.tile([128, 8 * 256], B16, tag="wv")
            wo_sb = pp.tile([128, 2 * 1024], B16, tag="wo")
            cos_sb = pp.tile([128, T], B16, tag="cos")
            sin_sb = pp.tile([128, T], B16, tag="sin")
            maskT_sb = pp.tile([128, 128], B16, tag="maskT")
            id_sb = pp.tile([128, 128], B16, tag="id")
            v_sb = pp.tile([128, 16 * HPC * 65], B16, tag="v")
            qk_rot = [pp.tile([128, T], B16, tag=f"rot{i}", name=f"rot{i}") for i in range(4)]
            aou = [pp.tile([128, T], B16, tag=f"aou{i}", name=f"aou{i}") for i in range(2)]

            # initial loads: interleave per k-chunk so accumulation chains
            # can begin as chunks arrive
            # batched loads, few DMA instructions (sync-queue issue rate is
            # ~0.65us/instruction), ordered so the first qk chains can start
            # after the first wqk/x half arrives
            xq_tiles = {}
            xq_tiles[0] = xqp.tile([128, 8 * QT], B16, tag="xq", name="xq0")
            xq_tiles[1] = xqp.tile([128, 8 * QT], B16, tag="xq", name="xq1")
            for kh in range(2):
                k0 = kh * 4
                nc.sync.dma_start(
                    wqk_sb[:, k0 * 512:(k0 + 4) * 512].rearrange("p (k c) -> p k c", k=4),
                    wqk[k0 * 128:(k0 + 4) * 128, :].rearrange("(k p) c -> p k c", p=128))
                nc.sync.dma_start(
                    xq_tiles[0][:, k0 * QT:(k0 + 4) * QT].rearrange("p (k c) -> p k c", k=4),
                    xT[k0 * 128:(k0 + 4) * 128, 0:QT].rearrange("(k p) c -> p k c", p=128))
            nc.sync.dma_start(wv_sb.rearrange("p (k c) -> p k c", k=8),
                              wv.rearrange("(k p) c -> p k c", p=128))
            nc.sync.dma_start(cos_sb[:], cosR[:])
            nc.sync.dma_start(sin_sb[:], sinS[:])
            nc.sync.dma_start(maskT_sb[:], maskT[:])
            nc.sync.dma_start(id_sb[:], ident[:])
            nc.sync.dma_start(xq_tiles[1].rearrange("p (k c) -> p k c", k=8),
                              xT[:, QT:T].rearrange("(k p) c -> p k c", p=128))
            nc.sync.dma_start(wo_sb.rearrange("p (k c) -> p k c", k=2),
                              wo.rearrange("(k p) c -> p k c", p=128))
            # ones column at col 64 of every v slot
            vcols = v_sb.rearrange("p (s d) -> p s d", d=65)
            nc.vector.memset(vcols[:, :, 64:65], 1.0)
            # SWDGE warmup: boot the gpsimd DMA path before the first
            # partition_broadcast sits on a critical chain
            wrm = rrp.tile([64, 64], B16, tag="wrm")
            nc.gpsimd.partition_broadcast(wrm[:], maskT_sb[0:1, 0:64])

            # ---------- QKV projection + RoPE ----------
            def qk_chains(q, ms):
                # paired m-chains with k-halves interleaved so accumulation
                # starts after the first wqk/x half-load lands
                t0 = q * QT
                xq_sb = xq_tiles[q]
                pss = {m: psA.tile([128, 1024], F32, tag="grp", name=f"qk{q}_{m}")
                       for m in ms}
                for kh in range(2):
                    for m in ms:
                        for n in range(2):
                            for k in range(kh * 4, kh * 4 + 4):
                                nc.tensor.matmul(
                                    pss[m][:, n * 512:(n + 1) * 512],
                                    wqk_sb[:, k * 512 + m * 128: k * 512 + (m + 1) * 128],
                                    xq_sb[:, k * QT + n * 512: k * QT + (n + 1) * 512],
                                    start=(k == 0), stop=(k == 7))
                for m in ms:
                    raw = rawp.tile([128, QT], B16, tag="raw")
                    nc.scalar.copy(raw[:], pss[m][:])
                    swp = swpp.tile([128, QT], B16, tag="swp")
                    nc.sync.dma_start(swp[0:32, :], raw[32:64, :])
                    nc.sync.dma_start(swp[32:64, :], raw[0:32, :])
                    nc.sync.dma_start(swp[64:96, :], raw[96:128, :])
                    nc.sync.dma_start(swp[96:128, :], raw[64:96, :])
                    rot = qk_rot[m]
                    tmp = swpp.tile([128, QT], B16, tag="tmp")
                    nc.vector.tensor_mul(tmp[:], swp[:], sin_sb[:, t0:t0 + QT])
                    nc.vector.tensor_mul(rot[:, t0:t0 + QT], raw[:], cos_sb[:, t0:t0 + QT])
                    nc.vector.tensor_add(rot[:, t0:t0 + QT], rot[:, t0:t0 + QT], tmp[:])

            def v_pair(q, j):
                # v for kb = q*8 + 2j, q*8 + 2j + 1
                xq_sb = xq_tiles[q]
                ps = psA.tile([128, 512], F32, tag="grp", name=f"v{q}_{j}")
                for i, mt in enumerate((2 * j, 2 * j + 1)):
                    for k in range(8):
                        nc.tensor.matmul(ps[:, i * 256:(i + 1) * 256],
                                         xq_sb[:, k * QT + mt * 128: k * QT + (mt + 1) * 128],
                                         wv_sb[:, k * 256:(k + 1) * 256],
                                         start=(k == 0), stop=(k == 7))
                vsrc = ps.rearrange("p (s h d) -> p s h d", s=2, h=4)
                base = (q * 8 + 2 * j) * 4 * 65
                vdst = v_sb[:, base:base + 2 * 4 * 65] \
                    .rearrange("p (s h e) -> p s h e", s=2, e=65)[:, :, :, 0:64]
                nc.scalar.copy(vdst, vsrc)

            # ---------- attention ----------
            def st_group(qb, h, g, st):
                h2 = h % 2
                Qt = qk_rot[0] if h < 2 else qk_rot[1]
                Kt = qk_rot[2] if h < 2 else qk_rot[3]
                r0 = 64 * h2
                for i in range(2):
                    kb = 2 * g + i
                    kt = Kt[r0:r0 + 64, kb * 128:(kb + 1) * 128]
                    if kb < 4 * qb:
                        nc.tensor.matmul(
                            st[:, i * 512:(i + 1) * 512], kt,
                            Qt[r0:r0 + 64, qb * 512:(qb + 1) * 512],
                            start=True, stop=True)
                    else:
                        dl = (kb - 4 * qb) * 128
                        if dl < 384:
                            nc.tensor.matmul(
                                st[:, i * 512 + dl + 128:(i + 1) * 512], kt,
                                Qt[r0:r0 + 64, qb * 512 + dl + 128:(qb + 1) * 512],
                                start=True, stop=True)
                        nc.tensor.matmul(
                            st[:, i * 512 + dl: i * 512 + dl + 128], kt,
                            Qt[r0:r0 + 64, qb * 512 + dl: qb * 512 + dl + 128],
                            start=True, stop=False)
                for i in range(2):
                    kb = 2 * g + i
                    if kb >= 4 * qb:
                        dl = (kb - 4 * qb) * 128
                        nc.tensor.matmul(
                            st[:, i * 512 + dl: i * 512 + dl + 128],
                            id_sb[:], maskT_sb[:],
                            start=False, stop=True)

            def pv_group(qb, h, g, pt, out_ps, live):
                for i in range(2):
                    kb = 2 * g + i
                    diag = kb >= 4 * qb
                    dl = (kb - 4 * qb) * 128 if diag else 0
                    nc.tensor.matmul(
                        out_ps[:, dl:512],
                        v_sb[:, (kb * 4 + h) * 65:(kb * 4 + h) * 65 + 65],
                        pt[:, i * 512 + dl:(i + 1) * 512],
                        start=(kb == 0), stop=(kb == live - 1))

            norm_state = {}

            def att_block(qb, hp, hpi):
                """Generator: one yield per g-step; ends after norm1."""
                live = 4 * (qb + 1)
                ng = live // 2
                out_ps = {}
                pts = {h: {} for h in hp}
                for g in range(ng + 1):
                    for h in hp:
                        if g < ng:
                            st = psA.tile([128, 1024], F32, tag="grp",
                                          name=f"st{qb}_{h}_{g}")
                            st_group(qb, h, g, st)
                            if g < 2 * qb:
                                ranges = ((0, 1024),)
                            elif g == 2 * qb:
                                ranges = ((0, 512), (640, 1024))
                            else:
                                ranges = ((256, 512), (896, 1024))
                            pt = ptp.tile([128, 1024], B16, tag="pt",
                                          name=f"pt{qb}_{h}_{g}")
                            for lo, hi in ranges:
                                nc.scalar.activation(pt[:, lo:hi], st[:, lo:hi],
                                                     AF.Exp, scale=0.125)
                            pts[h][g] = pt
                    for h in hp:
                        if g >= 1:
                            if h not in out_ps:
                                out_ps[h] = psO.tile([65, 512], F32, tag="acc",
                                                     name=f"ops{qb}_{h}")
                            pv_group(qb, h, g - 1, pts[h].pop(g - 1),
                                     out_ps[h], live)
                    if g < ng:
                        yield
                # norm part 1: stage both heads side by side; one reciprocal
                stage = stagep.tile([65, 1024], F32, tag="stage",
                                    name=f"stage{qb}_{hpi}")
                nc.vector.tensor_copy(stage[:, 0:512], out_ps[hp[0]][:])
                nc.vector.tensor_copy(stage[:, 512:1024], out_ps[hp[1]][:])
                den0 = rrp.tile([1, 1024], F32, tag="den", name=f"den{qb}_{hpi}")
                nc.sync.dma_start(den0[:], stage[64:65, :])
                rr = rrp.tile([1, 1024], F32, tag="rr", name=f"rr{qb}_{hpi}")
                with nc.allow_low_precision(reason="softmax denominators"):
                    nc.vector.reciprocal_approx_fast(rr[:], den0[:])
                rrb = rrp.tile([64, 1024], F32, tag="rrb", name=f"rrb{qb}_{hpi}")
                nc.gpsimd.partition_broadcast(rrb[:], rr[0:1, :])
                if debug and qb == 0 and hpi == 0:
                    nc.sync.dma_start(d_stage[:], stage[:])
                    nc.sync.dma_start(d_rr[:], rr[:])
                    nc.sync.dma_start(d_rrr[:], rrb[0:1, :])
                norm_state[(qb, hpi)] = (stage, rrb)
                yield

            def norm2(qb, hpi):
                stage, rrb = norm_state.pop((qb, hpi))
                AO = aou[hpi]
                nc.vector.tensor_mul(AO[0:64, qb * 512:(qb + 1) * 512],
                                     stage[0:64, 0:512], rrb[:, 0:512])
                nc.vector.tensor_mul(AO[64:128, qb * 512:(qb + 1) * 512],
                                     stage[0:64, 512:1024], rrb[:, 512:1024])

            def outproj_qt(qt):
                yp = psA.tile([128, 1024], F32, tag="grp", name=f"yp{qt}")
                for nh in range(2):
                    nc.tensor.matmul(yp[:, nh * 512:(nh + 1) * 512],
                                     aou[0][:, qt * 128:(qt + 1) * 128],
                                     wo_sb[:, nh * 512:(nh + 1) * 512],
                                     start=True, stop=False)
                    nc.tensor.matmul(yp[:, nh * 512:(nh + 1) * 512],
                                     aou[1][:, qt * 128:(qt + 1) * 128],
                                     wo_sb[:, 1024 + nh * 512:1024 + (nh + 1) * 512],
                                     start=False, stop=True)
                ysb = ysbp.tile([128, 1024], B16, tag="y", name=f"ysb{qt}")
                nc.vector.tensor_copy(ysb[:], yp[:])
                nc.sync.dma_start(out[qt * 128:(qt + 1) * 128, :], ysb[:])

            # ================= schedule =================
            # --- phase 1: QKV half 0, attention qb=1 hp0 interleaved;
            # all four rope chains early so nothing downstream waits on them
            gen0 = att_block(1, (0, 1), 0)
            qk_chains(0, (0, 2))
            qk_chains(0, (1, 3))
            for j in range(4):
                v_pair(0, j)
                next(gen0, None)
            for _ in gen0:
                pass

            # --- phase 2: QKV half 1, attention qb=1 hp1 interleaved
            gen1 = att_block(1, (2, 3), 1)
            qk_chains(1, (0, 2))
            next(gen1, None)
            qk_chains(1, (1, 3))
            next(gen1, None)
            for j in range(3):
                v_pair(1, j)
                next(gen1, None)
            v_pair(1, 3)

            # --- remaining query blocks: 3, 2, 0; outproj/norm2 one qb late
            def run_block(qb, fillers):
                # delay fillers a couple of g-steps so the bc matmul (which
                # waits on the previous block's reciprocal chain) never heads
                # the PE queue while early st-groups could run
                fillers = list(fillers)
                total = 2 * (2 * (qb + 1) + 1)
                skip = max(0, min(2, total - len(fillers)))
                step = 0
                for hpi, hp in enumerate(((0, 1), (2, 3))):
                    for _ in att_block(qb, hp, hpi):
                        if fillers and step >= skip:
                            fillers.pop(0)()
                        step += 1
                for f in fillers:
                    f()

            def mk_fillers(pqb):
                fs = [lambda: norm2(pqb, 0), lambda: norm2(pqb, 1)]
                fs += [(lambda qt=qt: outproj_qt(qt)) for qt in range(4 * pqb, 4 * pqb + 4)]
                return fs

            run_block(3, mk_fillers(1))
            run_block(2, mk_fillers(3))
            # qb0 + tail, hand-scheduled: norm2(0,0) lands inside the hp1
            # block; outproj(0) is split so its aou[0] halves run while the
            # hp1 normalization chain completes
            f2 = mk_fillers(2)
            for _ in att_block(0, (0, 1), 0):
                if f2:
                    f2.pop(0)()
            gtail = att_block(0, (2, 3), 1)
            next(gtail, None)
            if f2:
                f2.pop(0)()
            next(gtail, None)
            norm2(0, 0)
            for _ in gtail:
                if f2:
                    f2.pop(0)()
            for f in f2:
                f()
            norm2(0, 1)
            yps = {}
            for qt in range(0, 3):
                yp = psA.tile([128, 1024], F32, tag="grp", name=f"yp{qt}")
                for nh in range(2):
                    nc.tensor.matmul(yp[:, nh * 512:(nh + 1) * 512],
                                     aou[0][:, qt * 128:(qt + 1) * 128],
                                     wo_sb[:, nh * 512:(nh + 1) * 512],
                                     start=True, stop=False)
                yps[qt] = yp
            for qt in range(0, 3):
                yp = yps[qt]
                for nh in range(2):
                    nc.tensor.matmul(yp[:, nh * 512:(nh + 1) * 512],
                                     aou[1][:, qt * 128:(qt + 1) * 128],
                                     wo_sb[:, 1024 + nh * 512:1024 + (nh + 1) * 512],
                                     start=False, stop=True)
                ysb = ysbp.tile([128, 1024], B16, tag="y", name=f"ysb{qt}")
                nc.vector.tensor_copy(ysb[:], yp[:])
                nc.sync.dma_start(out[qt * 128:(qt + 1) * 128, :], ysb[:])
            outproj_qt(3)
            if debug:
                for i in range(4):
                    nc.sync.dma_start(d_rot[i][:], qk_rot[i][:])
                nc.sync.dma_start(d_v[:], v_sb[:])
                for i in range(2):
                    nc.sync.dma_start(d_aou[i][:], aou[i][:])

    nc.compile()
    return nc


def _core_inputs(x, cos, sin, W_qkv, W_out, core):
    b = core // 4
    hg = core % 4
    heads = list(range(4 * hg, 4 * hg + 4))

    xT = np.ascontiguousarray(x[b].T).astype(BF)
    qrows = np.concatenate([W_qkv[h * 64:(h + 1) * 64] for h in heads], 0)
    krows = np.concatenate([W_qkv[C + h * 64: C + (h + 1) * 64] for h in heads], 0)
    wqk = np.ascontiguousarray(np.concatenate([qrows, krows], 0).T).astype(BF)
    vrows = np.concatenate([W_qkv[2 * C + h * 64: 2 * C + (h + 1) * 64] for h in heads], 0)
    wv = np.ascontiguousarray(vrows.T).astype(BF)
    cols = np.concatenate([np.arange(h * 64, (h + 1) * 64) for h in heads])
    wo = np.ascontiguousarray(W_out[:, cols].T).astype(BF)

    cT = np.ascontiguousarray(cos.T)      # (32, T)
    sT = np.ascontiguousarray(sin.T)
    cosR = np.tile(cT, (4, 1)).astype(BF)
    sinS = np.concatenate([-sT, sT, -sT, sT], 0).astype(BF)

    p = np.arange(128)[:, None]
    j = np.arange(128)[None, :]
    maskT = np.where(p <= j, 0.0, NEG).astype(BF)

    return {
        "xT": xT, "wqk": wqk, "wv": wv, "wo": wo,
        "cosR": cosR, "sinS": sinS,
        "maskT": np.ascontiguousarray(maskT),
        "ident": np.eye(128).astype(BF),
    }


def kernel(x, cos, sin, mask, W_qkv, W_out):
    from concourse import bass_utils

    x = np.asarray(x, dtype=np.float32)
    cos = np.asarray(cos, dtype=np.float32)
    sin = np.asarray(sin, dtype=np.float32)
    W_qkv = np.asarray(W_qkv, dtype=np.float32)
    W_out = np.asarray(W_out, dtype=np.float32)

    if "nc" not in _cache:
        _cache["nc"] = _build()
    nc = _cache["nc"]

    in_maps = [_core_inputs(x, cos, sin, W_qkv, W_out, c) for c in range(NCORES)]
    res = bass_utils.run_bass_kernel_spmd(nc, in_maps, core_ids=list(range(NCORES)))

    y = np.zeros((B, T, C), dtype=np.float32)
    for c in range(NCORES):
        y[c // 4] += res.results[c]["out"].astype(np.float32)
    return y
